# revision 28
# baseline (speedup 1.0000x reference)
"""Trainium2 Bass kernel for nn_ArcPredictorWloss.

Reference computation (per sample s of n=16, l=256, h=hid=128):
  scores = tanh(f.reshape(l*l, h) @ W1 + b1) @ W2 + b2          # (l, l)
  C[i,j] = sum_h f[i,j,h] * f[j,i,h]   (symmetric)
  Cn = C / (max|C| + eps)
  r = softmax(scores + eps, axis=-1);  c = (head+eps)/sum(head+eps, -1)
  per-row Sinkhorn (20 iters) with K = exp(-20*Cn), KM = K*Cn
  wloss = sum over rows/samples of u . (KM v)

Sharding: data-parallel over n across 8 cores (2 samples per core).
All heavy compute in bf16 (validated: scores err ~3.5e-3 rel-to-max,
wloss err ~1.2e-3 worst-case vs fp64), fp32 PSUM accumulation.

Layout strategy per sample (l=256 -> 2x2 grid of 128x128 blocks):
  - A(r,c)[p,jj,h] = f[128r+p, 128c+jj, h]   (natural, row-partition)
  - B(r,c)[p,jj,h] = f[128c+jj, 128r+p, h]   (partner, row-partition)
    Both loaded straight from HBM by casting gpsimd DMAs (fp32->bf16).
    Loaded: A00,B00,A01,B01,A11,B11 - block10's bytes arrive as B01, so
    off-diagonal data is read once; diagonal blocks are read twice.
  - C(r,c) = sum_h A(r,c)*B(r,c): multiply on GPSIMD (otherwise idle),
    reduce on DVE.  C(1,0) = C(0,1)^T via PE transpose.
  - MLP: PE-transpose of [128 rows, h] col-tiles -> Xt, stage-1 vs W1,
    tanh on ACT (PSUM->SBUF), stage-2 as [arcs,1] columns (lhsT=hdn
    tile, rhs=W2) accumulated into [128,32] PSUM chunks -> scores in
    natural layout (block10 via B01 lands transposed; one extra PE
    transpose fixes it up).
  - softmax over free dim; Sinkhorn state transposed [bin, row]; K/KM
    symmetric so the loop is matmul + approx-reciprocal + multiply.
    The +EPS inside the loop is a rank-1 (eps-row x ones-row)
    PSUM-accumulated matmul.  Both samples' Sinkhorn chains are emitted
    interleaved so the two dependency chains overlap on the engines.
"""

import os
import sys
import numpy as np

sys.path.insert(0, "/opt/trn_rl_repo")

LAM = 20.0
N_ITERS = 20
EPS = 1e-8

N, L, H = 16, 256, 128
N_CORES = 8
S_PER_CORE = N // N_CORES  # 2


def build_nc():
    import concourse.bass as bass
    import concourse.mybir as mybir
    from concourse import bacc, masks
    from concourse.tile import TileContext

    f32 = mybir.dt.float32
    bf16 = mybir.dt.bfloat16
    AF = mybir.ActivationFunctionType
    ALU = mybir.AluOpType
    AX = mybir.AxisListType

    nc = bacc.Bacc("TRN2", target_bir_lowering=False)

    f_in = nc.declare_dram_parameter("f", [S_PER_CORE, L, L, H], f32, isOutput=False)
    head_in = nc.declare_dram_parameter("head", [S_PER_CORE, L, L], f32, isOutput=False)
    w1_in = nc.declare_dram_parameter("W1", [H, H], f32, isOutput=False)
    b1_in = nc.declare_dram_parameter("b1", [H], f32, isOutput=False)
    w2_in = nc.declare_dram_parameter("W2", [H, 1], f32, isOutput=False)
    b2_in = nc.declare_dram_parameter("b2", [1], f32, isOutput=False)
    scores_out = nc.declare_dram_parameter(
        "scores", [S_PER_CORE, L, L], f32, isOutput=True
    )
    wloss_out = nc.declare_dram_parameter("wloss", [1, 1], f32, isOutput=True)

    with TileContext(nc) as tc:
        from contextlib import ExitStack

        ctx = ExitStack()
        with ctx:
            const_pool = ctx.enter_context(tc.tile_pool(name="const", bufs=1))
            blk_pool = ctx.enter_context(tc.tile_pool(name="blk", bufs=4))
            xt_pool = ctx.enter_context(tc.tile_pool(name="xt", bufs=5))
            hdn_pool = ctx.enter_context(tc.tile_pool(name="hdn", bufs=5))
            prod_pool = ctx.enter_context(tc.tile_pool(name="prod", bufs=3))
            small_pool = ctx.enter_context(tc.tile_pool(name="small", bufs=2))
            samp_pool = ctx.enter_context(tc.tile_pool(name="samp", bufs=2))
            ps_xt = ctx.enter_context(tc.tile_pool(name="ps_xt", bufs=3, space="PSUM"))
            ps_mm = ctx.enter_context(tc.tile_pool(name="ps_mm", bufs=3, space="PSUM"))
            ps_s2 = ctx.enter_context(tc.tile_pool(name="ps_s2", bufs=2, space="PSUM"))

            # ---- constants / weights ----
            ident_bf = const_pool.tile([128, 128], bf16)
            masks.make_identity(nc, ident_bf[:])
            ident_f32 = const_pool.tile([128, 128], f32)
            masks.make_identity(nc, ident_f32[:])
            ones_f32 = const_pool.tile([128, 1], f32)
            nc.vector.memset(ones_f32[:], 1.0)
            ones_row_f32 = const_pool.tile([1, 128], f32)
            nc.vector.memset(ones_row_f32[:], 1.0)
            eps_row_bf = const_pool.tile([1, 128], bf16)
            nc.vector.memset(eps_row_bf[:], EPS)
            ones_row_bf = const_pool.tile([1, L], bf16)
            nc.vector.memset(ones_row_bf[:], 1.0)

            w1_f32 = const_pool.tile([H, H], f32)
            nc.sync.dma_start(out=w1_f32[:], in_=w1_in[:, :])
            w1_bf = const_pool.tile([H, H], bf16)
            nc.vector.tensor_copy(w1_bf[:], w1_f32[:])

            b1_sb = const_pool.tile([H, 1], f32)
            nc.sync.dma_start(
                out=b1_sb[:], in_=b1_in[:].rearrange("(h one) -> h one", one=1)
            )

            w2_f32 = const_pool.tile([H, 1], f32)
            nc.sync.dma_start(out=w2_f32[:], in_=w2_in[:, :])
            w2_bf = const_pool.tile([H, 1], bf16)
            nc.vector.tensor_copy(w2_bf[:], w2_f32[:])

            b2_sb = const_pool.tile([1, 1], f32)
            nc.sync.dma_start(
                out=b2_sb[:], in_=b2_in[:].rearrange("(o one) -> o one", one=1)
            )
            b2_bcast = const_pool.tile([128, 1], f32)
            nc.sync.dma_start(
                out=b2_bcast[:],
                in_=b2_in[:]
                .rearrange("(o one) -> o one", one=1)
                .to_broadcast((128, 1)),
            )

            # per-(sample,half) loss partials
            lossc = const_pool.tile([128, 2 * S_PER_CORE], f32)
            sink_state = []

            for s in range(S_PER_CORE):
                # ============ head -> ct (transposed target hist) ==========
                head_nat = [
                    samp_pool.tile([128, L], f32, tag="head", name=f"head_{s}_{i}")
                    for i in range(2)
                ]
                c_nat = [
                    samp_pool.tile([128, L], bf16, tag="cnat", name=f"cnat_{s}_{i}")
                    for i in range(2)
                ]
                for t in range(2):
                    nc.sync.dma_start(
                        out=head_nat[t][:], in_=head_in[s, 128 * t : 128 * (t + 1), :]
                    )
                    rs = small_pool.tile([128, 1], f32, tag="rs", name=f"rs_{s}_{t}")
                    nc.vector.tensor_reduce(rs[:], head_nat[t][:], axis=AX.X, op=ALU.add)
                    rs_eps = small_pool.tile(
                        [128, 1], f32, tag="rs_eps", name=f"rse_{s}_{t}"
                    )
                    nc.vector.tensor_scalar_add(rs_eps[:], rs[:], float(L) * EPS)
                    rec = small_pool.tile([128, 1], f32, tag="rec", name=f"rec_{s}_{t}")
                    nc.vector.reciprocal(rec[:], rs_eps[:])
                    # c = (head + eps) * (1/sum) in one DVE pass
                    nc.vector.tensor_scalar(
                        out=c_nat[t][:],
                        in0=head_nat[t][:],
                        scalar1=EPS,
                        scalar2=rec[:],
                        op0=ALU.add,
                        op1=ALU.mult,
                    )
                ct = [
                    samp_pool.tile([128, L], bf16, tag="ct", bufs=4, name=f"ct_{s}_{i}")
                    for i in range(2)
                ]
                for jt in range(2):
                    for it in range(2):
                        tp = ps_xt.tile(
                            [128, 128], bf16, tag="xt", name=f"tp_{s}_{jt}_{it}"
                        )
                        nc.tensor.transpose(
                            tp[:], c_nat[it][:, 128 * jt : 128 * (jt + 1)], ident_bf[:]
                        )
                        nc.vector.tensor_copy(ct[jt][:, 128 * it : 128 * (it + 1)], tp[:])

                # ============ stream f: MLP + C ============================
                scores_nat = [
                    samp_pool.tile([128, L], f32, tag="snat", name=f"snat_{s}_{i}")
                    for i in range(2)
                ]
                C_t = [
                    samp_pool.tile([128, L], bf16, tag="C", name=f"C_{s}_{i}")
                    for i in range(2)
                ]

                def load_A(r, c):
                    t = blk_pool.tile(
                        [128, 128, H], bf16, tag="blk", name=f"A_{s}_{r}_{c}"
                    )
                    nc.gpsimd.dma_start(
                        out=t[:, :, :],
                        in_=f_in[s, 128 * r : 128 * (r + 1), 128 * c : 128 * (c + 1), :],
                    )
                    return t

                def load_B(r, c):
                    # B[p, jj, h] = f[128c+jj, 128r+p, h] (strided direct
                    # load; split in jj-halves to stay under the 16384
                    # descriptor-per-DMA limit)
                    t = blk_pool.tile(
                        [128, 128, H], bf16, tag="blk", name=f"B_{s}_{r}_{c}"
                    )
                    for jh in range(2):
                        src = f_in[
                            s,
                            128 * c + 64 * jh : 128 * c + 64 * (jh + 1),
                            128 * r : 128 * (r + 1),
                            :,
                        ].rearrange("j p h -> p j h")
                        nc.gpsimd.dma_start(
                            out=t[:, 64 * jh : 64 * (jh + 1), :], in_=src
                        )
                    return t

                def shuffle_B_diag(src_tile, r):
                    # diagonal partner layout from the already-loaded A(r,r):
                    # per-row SBUF->SBUF DMAs on the otherwise-idle HWDGE,
                    # saving the 8 MiB HBM re-read of the block
                    t = blk_pool.tile(
                        [128, 128, H], bf16, tag="blk", name=f"B_{s}_{r}_{r}"
                    )
                    for jj in range(128):
                        nc.sync.dma_start(
                            out=t[:, jj, :], in_=src_tile[jj : jj + 1, :, :]
                        )
                    return t

                def mlp_block(blk, rt, cl, transposed_out=None):
                    # blk[:, jj, :] = 128 arcs; for A(r,c) these are
                    # (row 128r+p, col 128c+jj) -> scores_nat[rt] col chunks.
                    # For B(0,1) (= block10 data) they are (row 128+jj, col p)
                    # -> transposed staging, fixed up by one PE transpose.
                    s2_ps = None
                    for g in range(32):
                        xt_ps = ps_xt.tile(
                            [128, 512], bf16, tag="xt", name=f"xtps_{s}_{rt}_{cl}_{g}"
                        )
                        for t in range(4):
                            jj = 4 * g + t
                            nc.tensor.transpose(
                                xt_ps[:, 128 * t : 128 * (t + 1)],
                                blk[:, jj, :],
                                ident_bf[:],
                            )
                        xt_sb = xt_pool.tile(
                            [128, 512], bf16, tag="xt_sb", name=f"xtsb_{s}_{rt}_{cl}_{g}"
                        )
                        if g % 2 == 0:
                            nc.vector.tensor_copy(xt_sb[:], xt_ps[:])
                        else:
                            nc.scalar.copy(xt_sb[:], xt_ps[:])
                        hdn_ps = ps_mm.tile(
                            [128, 512], f32, tag="hdn", name=f"hdnps_{s}_{rt}_{cl}_{g}"
                        )
                        nc.tensor.matmul(hdn_ps[:], w1_bf[:], xt_sb[:])
                        hdn_sb = hdn_pool.tile(
                            [128, 512], bf16, tag="hdn_sb",
                            name=f"hdnsb_{s}_{rt}_{cl}_{g}",
                        )
                        nc.scalar.activation(hdn_sb[:], hdn_ps[:], AF.Tanh, bias=b1_sb[:])
                        if g % 8 == 0:
                            s2_ps = ps_s2.tile(
                                [128, 32], f32, tag="s2", name=f"s2ps_{s}_{rt}_{cl}_{g}"
                            )
                        for t in range(4):
                            jj = 4 * g + t
                            nc.tensor.matmul(
                                s2_ps[:, jj % 32 : jj % 32 + 1],
                                hdn_sb[:, 128 * t : 128 * (t + 1)],
                                w2_bf[:],
                            )
                        if g % 8 == 7:
                            q = g // 8
                            if transposed_out is None:
                                nc.vector.tensor_scalar_add(
                                    scores_nat[rt][
                                        :, 128 * cl + 32 * q : 128 * cl + 32 * (q + 1)
                                    ],
                                    s2_ps[:],
                                    b2_bcast[:],
                                )
                            else:
                                nc.vector.tensor_copy(
                                    transposed_out[:, 32 * q : 32 * (q + 1)], s2_ps[:]
                                )

                def c_compute(rt, col0, A, B):
                    for q in range(8):
                        sl = slice(16 * q, 16 * (q + 1))
                        prod = prod_pool.tile(
                            [128, 16, H], bf16, tag="prod",
                            name=f"prod_{s}_{rt}_{col0}_{q}",
                        )
                        nc.gpsimd.tensor_mul(prod[:], A[:, sl, :], B[:, sl, :])
                        with nc.allow_low_precision(
                            "C in bf16 validated: wloss err ~1e-3"
                        ):
                            nc.vector.tensor_reduce(
                                C_t[rt][:, col0 + 16 * q : col0 + 16 * (q + 1)],
                                prod[:],
                                axis=AX.X,
                                op=ALU.add,
                            )

                # ---- (0,0) ----
                A00 = load_A(0, 0)
                B00 = shuffle_B_diag(A00, 0)
                mlp_block(A00, 0, 0)
                c_compute(0, 0, A00, B00)
                # ---- (0,1) + (1,0) ----
                A01 = load_A(0, 1)
                B01 = load_B(0, 1)
                mlp_block(A01, 0, 1)
                sT10 = samp_pool.tile([128, 128], f32, tag="sT10", name=f"sT10_{s}")
                mlp_block(B01, 1, 0, transposed_out=sT10)
                tpS = ps_xt.tile([128, 128], f32, tag="xt", name=f"tpS_{s}")
                nc.tensor.transpose(tpS[:], sT10[:], ident_f32[:])
                nc.vector.tensor_scalar_add(scores_nat[1][:, 0:128], tpS[:], b2_bcast[:])
                c_compute(0, 128, A01, B01)
                # C(1,0) = C(0,1)^T
                tp_c = ps_xt.tile([128, 128], bf16, tag="xt", name=f"tpc_{s}")
                nc.tensor.transpose(tp_c[:], C_t[0][:, 128:256], ident_bf[:])
                nc.vector.tensor_copy(C_t[1][:, 0:128], tp_c[:])
                # ---- (1,1) ----
                A11 = load_A(1, 1)
                B11 = shuffle_B_diag(A11, 1)
                mlp_block(A11, 1, 1)
                c_compute(1, 128, A11, B11)

                # ============ scores output ================================
                for it in range(2):
                    nc.sync.dma_start(
                        out=scores_out[s, 128 * it : 128 * (it + 1), :],
                        in_=scores_nat[it][:],
                    )

                # ============ softmax (natural) + transpose to Rt ==========
                R_nat = [
                    samp_pool.tile([128, L], bf16, tag="Rnat", name=f"Rnat_{s}_{i}")
                    for i in range(2)
                ]
                for it in range(2):
                    mxr = small_pool.tile([128, 1], f32, tag="mxr", name=f"mxr_{s}_{it}")
                    nc.vector.tensor_reduce(
                        mxr[:], scores_nat[it][:], axis=AX.X, op=ALU.max
                    )
                    nmxr = small_pool.tile([128, 1], f32, tag="nmxr", name=f"nmxr_{s}_{it}")
                    nc.vector.tensor_scalar_mul(nmxr[:], mxr[:], -1.0)
                    e_nat = small_pool.tile([128, L], f32, tag="enat", name=f"enat_{s}_{it}")
                    zs = small_pool.tile([128, 1], f32, tag="zs", name=f"zs_{s}_{it}")
                    nc.scalar.activation(
                        e_nat[:], scores_nat[it][:], AF.Exp, bias=nmxr[:], accum_out=zs[:]
                    )
                    zrec = small_pool.tile([128, 1], f32, tag="zrec", name=f"zrec_{s}_{it}")
                    nc.vector.reciprocal(zrec[:], zs[:])
                    nc.vector.tensor_scalar_mul(R_nat[it][:], e_nat[:], zrec[:])
                Rt = [
                    samp_pool.tile([128, L], bf16, tag="Rt", bufs=4, name=f"Rt_{s}_{i}")
                    for i in range(2)
                ]
                for jt in range(2):
                    for it in range(2):
                        tpr = ps_xt.tile(
                            [128, 128], bf16, tag="xt", name=f"tpr_{s}_{jt}_{it}"
                        )
                        nc.tensor.transpose(
                            tpr[:], R_nat[it][:, 128 * jt : 128 * (jt + 1)], ident_bf[:]
                        )
                        nc.vector.tensor_copy(Rt[jt][:, 128 * it : 128 * (it + 1)], tpr[:])

                # ============ K, KM ========================================
                mx = small_pool.tile([128, 1], f32, tag="mx", name=f"mx_{s}")
                mx2 = small_pool.tile([128, 1], f32, tag="mx2", name=f"mx2_{s}")
                nc.vector.tensor_reduce(
                    mx[:], C_t[0][:], axis=AX.X, op=ALU.max, apply_absolute_value=True
                )
                nc.vector.tensor_reduce(
                    mx2[:], C_t[1][:], axis=AX.X, op=ALU.max, apply_absolute_value=True
                )
                mxc = small_pool.tile([128, 1], f32, tag="mxc", name=f"mxc_{s}")
                nc.vector.tensor_max(mxc[:], mx[:], mx2[:])
                mxt = ps_xt.tile([1, 128], f32, tag="xt", name=f"mxt_{s}")
                nc.tensor.transpose(mxt[:], mxc[:], ident_f32[:])
                mxs = small_pool.tile([1, 1], f32, tag="mxs", name=f"mxs_{s}")
                nc.vector.tensor_reduce(mxs[:], mxt[:], axis=AX.X, op=ALU.max)
                mxe = small_pool.tile([1, 1], f32, tag="mxe", name=f"mxe_{s}")
                nc.vector.tensor_scalar_add(mxe[:], mxs[:], EPS)
                inv = small_pool.tile([1, 1], f32, tag="inv", name=f"inv_{s}")
                nc.vector.reciprocal(inv[:], mxe[:])
                sk = small_pool.tile([1, 1], f32, tag="sk", name=f"sk_{s}")
                nc.scalar.mul(sk[:], inv[:], -LAM)
                inv_b = small_pool.tile([128, 1], f32, tag="inv_b", name=f"invb_{s}")
                inv_ps = ps_xt.tile([128, 1], f32, tag="xt", name=f"invps_{s}")
                nc.tensor.matmul(inv_ps[:], ones_row_f32[:], inv[:])
                nc.vector.tensor_copy(inv_b[:], inv_ps[:])
                sk_b = small_pool.tile([128, 1], f32, tag="sk_b", name=f"skb_{s}")
                sk_ps = ps_xt.tile([128, 1], f32, tag="xt", name=f"skps2_{s}")
                nc.tensor.matmul(sk_ps[:], ones_row_f32[:], sk[:])
                nc.vector.tensor_copy(sk_b[:], sk_ps[:])

                K_t = [
                    samp_pool.tile([128, L], bf16, tag="K", bufs=4, name=f"K_{s}_{i}")
                    for i in range(2)
                ]
                KM_t = [
                    samp_pool.tile([128, L], bf16, tag="KM", bufs=4, name=f"KM_{s}_{i}")
                    for i in range(2)
                ]
                for t in range(2):
                    nc.scalar.activation(K_t[t][:], C_t[t][:], AF.Exp, scale=sk_b[:])
                    nc.vector.scalar_tensor_tensor(
                        out=KM_t[t][:],
                        in0=K_t[t][:],
                        scalar=inv_b[:],
                        in1=C_t[t][:],
                        op0=ALU.mult,
                        op1=ALU.mult,
                    )

                # ============ Sinkhorn state (loop runs after both samples) =
                Ut = [
                    samp_pool.tile([128, L], bf16, tag="Ut", bufs=4, name=f"Ut_{s}_{i}")
                    for i in range(2)
                ]
                for t in range(2):
                    nc.vector.memset(Ut[t][:], 1.0 / L)
                Bt = [
                    samp_pool.tile([128, L], bf16, tag="Bt", bufs=4, name=f"Bt_{s}_{i}")
                    for i in range(2)
                ]
                sink_state.append(dict(ct=ct, Rt=Rt, K_t=K_t, KM_t=KM_t, Ut=Ut, Bt=Bt))

            # ============ Sinkhorn: both samples interleaved ================
            def half_step(s, dst_name, numer_name, it_tag):
                st = sink_state[s]
                dst, numer = st[dst_name], st[numer_name]
                rhs_tiles = st["Ut"] if dst_name == "Bt" else st["Bt"]
                lhs_tiles = st["K_t"]
                for hf in range(2):
                    ps = ps_mm.tile(
                        [128, L], f32, tag="hdn", name=f"skps_{s}_{it_tag}_{hf}"
                    )
                    nc.tensor.matmul(
                        ps[:], eps_row_bf[:], ones_row_bf[:], start=True, stop=False
                    )
                    nc.tensor.matmul(
                        ps[:],
                        lhs_tiles[0][:, 128 * hf : 128 * (hf + 1)],
                        rhs_tiles[0][:],
                        start=False,
                        stop=False,
                    )
                    nc.tensor.matmul(
                        ps[:],
                        lhs_tiles[1][:, 128 * hf : 128 * (hf + 1)],
                        rhs_tiles[1][:],
                        start=False,
                        stop=True,
                    )
                    rcp = small_pool.tile(
                        [128, L], f32, tag="rcp", name=f"rcp_{s}_{it_tag}_{hf}", bufs=4
                    )
                    nc.vector.reciprocal_approx_fast(rcp[:], ps[:])
                    nc.vector.tensor_mul(dst[hf][:], numer[hf][:], rcp[:])

            for it_ in range(N_ITERS):
                for s in range(S_PER_CORE):
                    half_step(s, "Bt", "ct", 2 * it_)
                for s in range(S_PER_CORE):
                    half_step(s, "Ut", "Rt", 2 * it_ + 1)

            for s in range(S_PER_CORE):
                half_step(s, "Bt", "ct", 99)  # Bt = Vt
            for s in range(S_PER_CORE):
                st = sink_state[s]
                KM_t, Bt, Ut = st["KM_t"], st["Bt"], st["Ut"]
                for hf in range(2):
                    ps = ps_mm.tile([128, L], f32, tag="hdn", name=f"pt_{s}_{hf}")
                    nc.tensor.matmul(
                        ps[:], KM_t[0][:, 128 * hf : 128 * (hf + 1)], Bt[0][:],
                        start=True, stop=False,
                    )
                    nc.tensor.matmul(
                        ps[:], KM_t[1][:, 128 * hf : 128 * (hf + 1)], Bt[1][:],
                        start=False, stop=True,
                    )
                    junk = small_pool.tile([128, L], bf16, tag="junk", name=f"junk_{s}_{hf}")
                    nc.vector.scalar_tensor_tensor(
                        out=junk[:],
                        in0=Ut[hf][:],
                        scalar=1.0,
                        in1=ps[:],
                        op0=ALU.mult,
                        op1=ALU.mult,
                        accum_out=lossc[:, 2 * s + hf : 2 * s + hf + 1],
                    )

            # ---- total wloss ----
            wl_ps = ps_s2.tile([1, 2 * S_PER_CORE], f32, tag="s2", name="wl_ps")
            nc.tensor.matmul(wl_ps[:], ones_f32[:], lossc[:])
            wl_sb = const_pool.tile([1, 1], f32)
            nc.vector.tensor_reduce(wl_sb[:], wl_ps[:], axis=AX.X, op=ALU.add)
            wl_out_sb = const_pool.tile([1, 1], f32)
            nc.vector.tensor_copy(wl_out_sb[:], wl_sb[:])
            nc.sync.dma_start(out=wloss_out[:, :], in_=wl_out_sb[:])

    nc.finalize()
    return nc


_NC_CACHE = None


def _get_nc():
    global _NC_CACHE
    if _NC_CACHE is None:
        _NC_CACHE = build_nc()
    return _NC_CACHE


LAST_EXEC_NS = None


def kernel(f, head, W1, b1, W2, b2):
    f = np.ascontiguousarray(f, dtype=np.float32)
    head = np.ascontiguousarray(head, dtype=np.float32)
    W1 = np.ascontiguousarray(W1, dtype=np.float32)
    b1 = np.ascontiguousarray(b1, dtype=np.float32)
    W2 = np.ascontiguousarray(W2, dtype=np.float32)
    b2 = np.ascontiguousarray(b2, dtype=np.float32)

    from concourse.bass_utils import run_bass_kernel_spmd

    nc = _get_nc()
    in_maps = []
    for cid in range(N_CORES):
        s0 = cid * S_PER_CORE
        in_maps.append(
            {
                "f": f[s0 : s0 + S_PER_CORE],
                "head": head[s0 : s0 + S_PER_CORE],
                "W1": W1,
                "b1": b1,
                "W2": W2,
                "b2": b2,
            }
        )
    trace = bool(int(os.environ.get("ARC_KERNEL_TRACE", "0")))
    res = run_bass_kernel_spmd(nc, in_maps, list(range(N_CORES)), trace=trace)
    if trace:
        global LAST_EXEC_NS
        LAST_EXEC_NS = res.exec_time_ns
    scores = np.concatenate(
        [np.asarray(r["scores"]).reshape(S_PER_CORE, L, L) for r in res.results],
        axis=0,
    )
    wloss = np.float32(
        sum(float(np.asarray(r["wloss"]).reshape(-1)[0]) for r in res.results)
    )
    return scores, np.asarray(wloss, dtype=np.float32)


# revision 30
# speedup vs baseline: 1.6205x; 1.6205x over previous
"""Trainium2 Bass kernel for nn_ArcPredictorWloss.

Reference computation (per sample s of n=16, l=256, h=hid=128):
  scores = tanh(f.reshape(l*l, h) @ W1 + b1) @ W2 + b2          # (l, l)
  C[i,j] = sum_h f[i,j,h] * f[j,i,h]   (symmetric)
  Cn = C / (max|C| + eps)
  r = softmax(scores + eps, axis=-1);  c = (head+eps)/sum(head+eps, -1)
  per-row Sinkhorn (20 iters) with K = exp(-20*Cn), KM = K*Cn
  wloss = sum over rows/samples of u . (KM v)

Sharding: data-parallel over n across 8 cores (2 samples per core).
All heavy compute in bf16 (validated: scores err ~3.5e-3 rel-to-max,
wloss err ~1.2e-3 worst-case vs fp64), fp32 PSUM accumulation.

Layout strategy per sample (l=256 -> 2x2 grid of 128x128 blocks):
  - A(r,c)[p,jj,h] = f[128r+p, 128c+jj, h]   (natural, row-partition)
  - B(r,c)[p,jj,h] = f[128c+jj, 128r+p, h]   (partner, row-partition)
    Both loaded straight from HBM by casting gpsimd DMAs (fp32->bf16).
    Loaded: A00,B00,A01,B01,A11,B11 - block10's bytes arrive as B01, so
    off-diagonal data is read once; diagonal blocks are read twice.
  - C(r,c) = sum_h A(r,c)*B(r,c): multiply on GPSIMD (otherwise idle),
    reduce on DVE.  C(1,0) = C(0,1)^T via PE transpose.
  - MLP: PE-transpose of [128 rows, h] col-tiles -> Xt, stage-1 vs W1,
    tanh on ACT (PSUM->SBUF), stage-2 as [arcs,1] columns (lhsT=hdn
    tile, rhs=W2) accumulated into [128,32] PSUM chunks -> scores in
    natural layout (block10 via B01 lands transposed; one extra PE
    transpose fixes it up).
  - softmax over free dim; Sinkhorn state transposed [bin, row]; K/KM
    symmetric so the loop is matmul + approx-reciprocal + multiply.
    The +EPS inside the loop is a rank-1 (eps-row x ones-row)
    PSUM-accumulated matmul.  Both samples' Sinkhorn chains are emitted
    interleaved so the two dependency chains overlap on the engines.
"""

import os
import sys
import numpy as np

sys.path.insert(0, "/opt/trn_rl_repo")

LAM = 20.0
N_ITERS = 20
EPS = 1e-8

N, L, H = 16, 256, 128
N_CORES = 8
S_PER_CORE = N // N_CORES  # 2


def build_nc():
    import concourse.bass as bass
    import concourse.mybir as mybir
    from concourse import bacc, masks
    from concourse.tile import TileContext

    f32 = mybir.dt.float32
    bf16 = mybir.dt.bfloat16
    AF = mybir.ActivationFunctionType
    ALU = mybir.AluOpType
    AX = mybir.AxisListType

    nc = bacc.Bacc("TRN2", target_bir_lowering=False)

    f_in = nc.declare_dram_parameter("f", [S_PER_CORE, L, L, H], f32, isOutput=False)
    head_in = nc.declare_dram_parameter("head", [S_PER_CORE, L, L], f32, isOutput=False)
    w1_in = nc.declare_dram_parameter("W1", [H, H], f32, isOutput=False)
    b1_in = nc.declare_dram_parameter("b1", [H], f32, isOutput=False)
    w2_in = nc.declare_dram_parameter("W2", [H, 1], f32, isOutput=False)
    b2_in = nc.declare_dram_parameter("b2", [1], f32, isOutput=False)
    scores_out = nc.declare_dram_parameter(
        "scores", [S_PER_CORE, L, L], f32, isOutput=True
    )
    wloss_out = nc.declare_dram_parameter("wloss", [1, 1], f32, isOutput=True)

    with TileContext(nc) as tc:
        from contextlib import ExitStack

        ctx = ExitStack()
        with ctx:
            const_pool = ctx.enter_context(tc.tile_pool(name="const", bufs=1))
            blk_pool = ctx.enter_context(tc.tile_pool(name="blk", bufs=4))
            xt_pool = ctx.enter_context(tc.tile_pool(name="xt", bufs=5))
            hdn_pool = ctx.enter_context(tc.tile_pool(name="hdn", bufs=5))
            prod_pool = ctx.enter_context(tc.tile_pool(name="prod", bufs=3))
            small_pool = ctx.enter_context(tc.tile_pool(name="small", bufs=2))
            samp_pool = ctx.enter_context(tc.tile_pool(name="samp", bufs=2))
            ps_xt = ctx.enter_context(tc.tile_pool(name="ps_xt", bufs=3, space="PSUM"))
            ps_mm = ctx.enter_context(tc.tile_pool(name="ps_mm", bufs=3, space="PSUM"))
            ps_s2 = ctx.enter_context(tc.tile_pool(name="ps_s2", bufs=2, space="PSUM"))

            # ---- constants / weights ----
            ident_bf = const_pool.tile([128, 128], bf16)
            masks.make_identity(nc, ident_bf[:])
            ident_f32 = const_pool.tile([128, 128], f32)
            masks.make_identity(nc, ident_f32[:])
            ones_f32 = const_pool.tile([128, 1], f32)
            nc.vector.memset(ones_f32[:], 1.0)
            ones_row_f32 = const_pool.tile([1, 128], f32)
            nc.vector.memset(ones_row_f32[:], 1.0)
            eps_row_bf = const_pool.tile([1, 128], bf16)
            nc.vector.memset(eps_row_bf[:], EPS)
            ones_row_bf = const_pool.tile([1, L], bf16)
            nc.vector.memset(ones_row_bf[:], 1.0)

            w1_f32 = const_pool.tile([H, H], f32)
            nc.sync.dma_start(out=w1_f32[:], in_=w1_in[:, :])
            w1_bf = const_pool.tile([H, H], bf16)
            nc.vector.tensor_copy(w1_bf[:], w1_f32[:])

            b1_sb = const_pool.tile([H, 1], f32)
            nc.sync.dma_start(
                out=b1_sb[:], in_=b1_in[:].rearrange("(h one) -> h one", one=1)
            )

            w2_f32 = const_pool.tile([H, 1], f32)
            nc.sync.dma_start(out=w2_f32[:], in_=w2_in[:, :])
            w2_bf = const_pool.tile([H, 1], bf16)
            nc.vector.tensor_copy(w2_bf[:], w2_f32[:])

            b2_sb = const_pool.tile([1, 1], f32)
            nc.sync.dma_start(
                out=b2_sb[:], in_=b2_in[:].rearrange("(o one) -> o one", one=1)
            )
            b2_bcast = const_pool.tile([128, 1], f32)
            nc.sync.dma_start(
                out=b2_bcast[:],
                in_=b2_in[:]
                .rearrange("(o one) -> o one", one=1)
                .to_broadcast((128, 1)),
            )

            # per-(sample,half) loss partials
            lossc = const_pool.tile([128, 2 * S_PER_CORE], f32)
            sink_state = []

            for s in range(S_PER_CORE):
                # ============ head -> ct (transposed target hist) ==========
                head_nat = [
                    samp_pool.tile([128, L], f32, tag="head", name=f"head_{s}_{i}")
                    for i in range(2)
                ]
                c_nat = [
                    samp_pool.tile([128, L], bf16, tag="cnat", name=f"cnat_{s}_{i}")
                    for i in range(2)
                ]
                for t in range(2):
                    nc.sync.dma_start(
                        out=head_nat[t][:], in_=head_in[s, 128 * t : 128 * (t + 1), :]
                    )
                    rs = small_pool.tile([128, 1], f32, tag="rs", name=f"rs_{s}_{t}")
                    nc.vector.tensor_reduce(rs[:], head_nat[t][:], axis=AX.X, op=ALU.add)
                    rs_eps = small_pool.tile(
                        [128, 1], f32, tag="rs_eps", name=f"rse_{s}_{t}"
                    )
                    nc.vector.tensor_scalar_add(rs_eps[:], rs[:], float(L) * EPS)
                    rec = small_pool.tile([128, 1], f32, tag="rec", name=f"rec_{s}_{t}")
                    nc.vector.reciprocal(rec[:], rs_eps[:])
                    # c = (head + eps) * (1/sum) in one DVE pass
                    nc.vector.tensor_scalar(
                        out=c_nat[t][:],
                        in0=head_nat[t][:],
                        scalar1=EPS,
                        scalar2=rec[:],
                        op0=ALU.add,
                        op1=ALU.mult,
                    )
                ct = [
                    samp_pool.tile([128, L], bf16, tag="ct", bufs=4, name=f"ct_{s}_{i}")
                    for i in range(2)
                ]
                for jt in range(2):
                    for it in range(2):
                        tp = ps_xt.tile(
                            [128, 128], bf16, tag="xt", name=f"tp_{s}_{jt}_{it}"
                        )
                        nc.tensor.transpose(
                            tp[:], c_nat[it][:, 128 * jt : 128 * (jt + 1)], ident_bf[:]
                        )
                        nc.vector.tensor_copy(ct[jt][:, 128 * it : 128 * (it + 1)], tp[:])

                # ============ stream f: MLP + C ============================
                scores_nat = [
                    samp_pool.tile([128, L], f32, tag="snat", name=f"snat_{s}_{i}")
                    for i in range(2)
                ]
                C_t = [
                    samp_pool.tile([128, L], bf16, tag="C", name=f"C_{s}_{i}")
                    for i in range(2)
                ]

                def load_A(r, c):
                    t = blk_pool.tile(
                        [128, 128, H], bf16, tag="blk", name=f"A_{s}_{r}_{c}"
                    )
                    nc.gpsimd.dma_start(
                        out=t[:, :, :],
                        in_=f_in[s, 128 * r : 128 * (r + 1), 128 * c : 128 * (c + 1), :],
                    )
                    return t

                def load_B(r, c):
                    # B[p, jj, h] = f[128c+jj, 128r+p, h] (strided direct
                    # load; split in jj-halves to stay under the 16384
                    # descriptor-per-DMA limit)
                    t = blk_pool.tile(
                        [128, 128, H], bf16, tag="blk", name=f"B_{s}_{r}_{c}"
                    )
                    for jh in range(2):
                        src = f_in[
                            s,
                            128 * c + 64 * jh : 128 * c + 64 * (jh + 1),
                            128 * r : 128 * (r + 1),
                            :,
                        ].rearrange("j p h -> p j h")
                        nc.gpsimd.dma_start(
                            out=t[:, 64 * jh : 64 * (jh + 1), :], in_=src
                        )
                    return t

                def shuffle_B_diag(src_tile, r):
                    # diagonal partner layout from the already-loaded A(r,r):
                    # per-row SBUF->SBUF DMAs on the otherwise-idle HWDGE,
                    # saving the 8 MiB HBM re-read of the block
                    t = blk_pool.tile(
                        [128, 128, H], bf16, tag="blk", name=f"B_{s}_{r}_{r}"
                    )
                    for jj in range(128):
                        nc.sync.dma_start(
                            out=t[:, jj, :], in_=src_tile[jj : jj + 1, :, :]
                        )
                    return t

                def mlp_block(blk, rt, cl, transposed_out=None):
                    # blk[:, jj, :] = 128 arcs; for A(r,c) these are
                    # (row 128r+p, col 128c+jj) -> scores_nat[rt] col chunks.
                    # For B(0,1) (= block10 data) they are (row 128+jj, col p)
                    # -> transposed staging, fixed up by one PE transpose.
                    s2_ps = None
                    for g in range(32):
                        xt_ps = ps_xt.tile(
                            [128, 512], bf16, tag="xt", name=f"xtps_{s}_{rt}_{cl}_{g}"
                        )
                        for t in range(4):
                            jj = 4 * g + t
                            nc.tensor.transpose(
                                xt_ps[:, 128 * t : 128 * (t + 1)],
                                blk[:, jj, :],
                                ident_bf[:],
                            )
                        xt_sb = xt_pool.tile(
                            [128, 512], bf16, tag="xt_sb", name=f"xtsb_{s}_{rt}_{cl}_{g}"
                        )
                        if g % 2 == 0:
                            nc.vector.tensor_copy(xt_sb[:], xt_ps[:])
                        else:
                            nc.scalar.copy(xt_sb[:], xt_ps[:])
                        hdn_ps = ps_mm.tile(
                            [128, 512], f32, tag="hdn", name=f"hdnps_{s}_{rt}_{cl}_{g}"
                        )
                        nc.tensor.matmul(hdn_ps[:], w1_bf[:], xt_sb[:])
                        hdn_sb = hdn_pool.tile(
                            [128, 512], bf16, tag="hdn_sb",
                            name=f"hdnsb_{s}_{rt}_{cl}_{g}",
                        )
                        nc.scalar.activation(hdn_sb[:], hdn_ps[:], AF.Tanh, bias=b1_sb[:])
                        if g % 8 == 0:
                            s2_ps = ps_s2.tile(
                                [128, 32], f32, tag="s2", name=f"s2ps_{s}_{rt}_{cl}_{g}"
                            )
                        for t in range(4):
                            jj = 4 * g + t
                            nc.tensor.matmul(
                                s2_ps[:, jj % 32 : jj % 32 + 1],
                                hdn_sb[:, 128 * t : 128 * (t + 1)],
                                w2_bf[:],
                            )
                        if g % 8 == 7:
                            q = g // 8
                            if transposed_out is None:
                                nc.vector.tensor_scalar_add(
                                    scores_nat[rt][
                                        :, 128 * cl + 32 * q : 128 * cl + 32 * (q + 1)
                                    ],
                                    s2_ps[:],
                                    b2_bcast[:],
                                )
                            else:
                                nc.vector.tensor_copy(
                                    transposed_out[:, 32 * q : 32 * (q + 1)], s2_ps[:]
                                )

                def c_compute(rt, col0, A, B):
                    for q in range(8):
                        sl = slice(16 * q, 16 * (q + 1))
                        prod = prod_pool.tile(
                            [128, 16, H], bf16, tag="prod",
                            name=f"prod_{s}_{rt}_{col0}_{q}",
                        )
                        nc.vector.tensor_mul(prod[:], A[:, sl, :], B[:, sl, :])
                        with nc.allow_low_precision(
                            "C in bf16 validated: wloss err ~1e-3"
                        ):
                            nc.vector.tensor_reduce(
                                C_t[rt][:, col0 + 16 * q : col0 + 16 * (q + 1)],
                                prod[:],
                                axis=AX.X,
                                op=ALU.add,
                            )

                # ---- (0,0) ----
                A00 = load_A(0, 0)
                B00 = load_B(0, 0)
                mlp_block(A00, 0, 0)
                c_compute(0, 0, A00, B00)
                # ---- (0,1) + (1,0) ----
                A01 = load_A(0, 1)
                B01 = load_B(0, 1)
                mlp_block(A01, 0, 1)
                sT10 = samp_pool.tile([128, 128], f32, tag="sT10", name=f"sT10_{s}")
                mlp_block(B01, 1, 0, transposed_out=sT10)
                tpS = ps_xt.tile([128, 128], f32, tag="xt", name=f"tpS_{s}")
                nc.tensor.transpose(tpS[:], sT10[:], ident_f32[:])
                nc.vector.tensor_scalar_add(scores_nat[1][:, 0:128], tpS[:], b2_bcast[:])
                c_compute(0, 128, A01, B01)
                # C(1,0) = C(0,1)^T
                tp_c = ps_xt.tile([128, 128], bf16, tag="xt", name=f"tpc_{s}")
                nc.tensor.transpose(tp_c[:], C_t[0][:, 128:256], ident_bf[:])
                nc.vector.tensor_copy(C_t[1][:, 0:128], tp_c[:])
                # ---- (1,1) ----
                A11 = load_A(1, 1)
                B11 = load_B(1, 1)
                mlp_block(A11, 1, 1)
                c_compute(1, 128, A11, B11)

                # ============ scores output ================================
                for it in range(2):
                    nc.sync.dma_start(
                        out=scores_out[s, 128 * it : 128 * (it + 1), :],
                        in_=scores_nat[it][:],
                    )

                # ============ softmax (natural) + transpose to Rt ==========
                R_nat = [
                    samp_pool.tile([128, L], bf16, tag="Rnat", name=f"Rnat_{s}_{i}")
                    for i in range(2)
                ]
                for it in range(2):
                    mxr = small_pool.tile([128, 1], f32, tag="mxr", name=f"mxr_{s}_{it}")
                    nc.vector.tensor_reduce(
                        mxr[:], scores_nat[it][:], axis=AX.X, op=ALU.max
                    )
                    nmxr = small_pool.tile([128, 1], f32, tag="nmxr", name=f"nmxr_{s}_{it}")
                    nc.vector.tensor_scalar_mul(nmxr[:], mxr[:], -1.0)
                    e_nat = small_pool.tile([128, L], f32, tag="enat", name=f"enat_{s}_{it}")
                    zs = small_pool.tile([128, 1], f32, tag="zs", name=f"zs_{s}_{it}")
                    nc.scalar.activation(
                        e_nat[:], scores_nat[it][:], AF.Exp, bias=nmxr[:], accum_out=zs[:]
                    )
                    zrec = small_pool.tile([128, 1], f32, tag="zrec", name=f"zrec_{s}_{it}")
                    nc.vector.reciprocal(zrec[:], zs[:])
                    nc.vector.tensor_scalar_mul(R_nat[it][:], e_nat[:], zrec[:])
                Rt = [
                    samp_pool.tile([128, L], bf16, tag="Rt", bufs=4, name=f"Rt_{s}_{i}")
                    for i in range(2)
                ]
                for jt in range(2):
                    for it in range(2):
                        tpr = ps_xt.tile(
                            [128, 128], bf16, tag="xt", name=f"tpr_{s}_{jt}_{it}"
                        )
                        nc.tensor.transpose(
                            tpr[:], R_nat[it][:, 128 * jt : 128 * (jt + 1)], ident_bf[:]
                        )
                        nc.vector.tensor_copy(Rt[jt][:, 128 * it : 128 * (it + 1)], tpr[:])

                # ============ K, KM ========================================
                mx = small_pool.tile([128, 1], f32, tag="mx", name=f"mx_{s}")
                mx2 = small_pool.tile([128, 1], f32, tag="mx2", name=f"mx2_{s}")
                nc.vector.tensor_reduce(
                    mx[:], C_t[0][:], axis=AX.X, op=ALU.max, apply_absolute_value=True
                )
                nc.vector.tensor_reduce(
                    mx2[:], C_t[1][:], axis=AX.X, op=ALU.max, apply_absolute_value=True
                )
                mxc = small_pool.tile([128, 1], f32, tag="mxc", name=f"mxc_{s}")
                nc.vector.tensor_max(mxc[:], mx[:], mx2[:])
                mxt = ps_xt.tile([1, 128], f32, tag="xt", name=f"mxt_{s}")
                nc.tensor.transpose(mxt[:], mxc[:], ident_f32[:])
                mxs = small_pool.tile([1, 1], f32, tag="mxs", name=f"mxs_{s}")
                nc.vector.tensor_reduce(mxs[:], mxt[:], axis=AX.X, op=ALU.max)
                mxe = small_pool.tile([1, 1], f32, tag="mxe", name=f"mxe_{s}")
                nc.vector.tensor_scalar_add(mxe[:], mxs[:], EPS)
                inv = small_pool.tile([1, 1], f32, tag="inv", name=f"inv_{s}")
                nc.vector.reciprocal(inv[:], mxe[:])
                sk = small_pool.tile([1, 1], f32, tag="sk", name=f"sk_{s}")
                nc.scalar.mul(sk[:], inv[:], -LAM)
                inv_b = small_pool.tile([128, 1], f32, tag="inv_b", name=f"invb_{s}")
                inv_ps = ps_xt.tile([128, 1], f32, tag="xt", name=f"invps_{s}")
                nc.tensor.matmul(inv_ps[:], ones_row_f32[:], inv[:])
                nc.vector.tensor_copy(inv_b[:], inv_ps[:])
                sk_b = small_pool.tile([128, 1], f32, tag="sk_b", name=f"skb_{s}")
                sk_ps = ps_xt.tile([128, 1], f32, tag="xt", name=f"skps2_{s}")
                nc.tensor.matmul(sk_ps[:], ones_row_f32[:], sk[:])
                nc.vector.tensor_copy(sk_b[:], sk_ps[:])

                K_t = [
                    samp_pool.tile([128, L], bf16, tag="K", bufs=4, name=f"K_{s}_{i}")
                    for i in range(2)
                ]
                KM_t = [
                    samp_pool.tile([128, L], bf16, tag="KM", bufs=4, name=f"KM_{s}_{i}")
                    for i in range(2)
                ]
                for t in range(2):
                    nc.scalar.activation(K_t[t][:], C_t[t][:], AF.Exp, scale=sk_b[:])
                    nc.vector.scalar_tensor_tensor(
                        out=KM_t[t][:],
                        in0=K_t[t][:],
                        scalar=inv_b[:],
                        in1=C_t[t][:],
                        op0=ALU.mult,
                        op1=ALU.mult,
                    )

                # ============ Sinkhorn state (loop runs after both samples) =
                Ut = [
                    samp_pool.tile([128, L], bf16, tag="Ut", bufs=4, name=f"Ut_{s}_{i}")
                    for i in range(2)
                ]
                for t in range(2):
                    nc.vector.memset(Ut[t][:], 1.0 / L)
                Bt = [
                    samp_pool.tile([128, L], bf16, tag="Bt", bufs=4, name=f"Bt_{s}_{i}")
                    for i in range(2)
                ]
                sink_state.append(dict(ct=ct, Rt=Rt, K_t=K_t, KM_t=KM_t, Ut=Ut, Bt=Bt))

            # ============ Sinkhorn: both samples interleaved ================
            def half_step(s, dst_name, numer_name, it_tag):
                st = sink_state[s]
                dst, numer = st[dst_name], st[numer_name]
                rhs_tiles = st["Ut"] if dst_name == "Bt" else st["Bt"]
                lhs_tiles = st["K_t"]
                for hf in range(2):
                    ps = ps_mm.tile(
                        [128, L], f32, tag="hdn", name=f"skps_{s}_{it_tag}_{hf}"
                    )
                    nc.tensor.matmul(
                        ps[:], eps_row_bf[:], ones_row_bf[:], start=True, stop=False
                    )
                    nc.tensor.matmul(
                        ps[:],
                        lhs_tiles[0][:, 128 * hf : 128 * (hf + 1)],
                        rhs_tiles[0][:],
                        start=False,
                        stop=False,
                    )
                    nc.tensor.matmul(
                        ps[:],
                        lhs_tiles[1][:, 128 * hf : 128 * (hf + 1)],
                        rhs_tiles[1][:],
                        start=False,
                        stop=True,
                    )
                    rcp = small_pool.tile(
                        [128, L], f32, tag="rcp", name=f"rcp_{s}_{it_tag}_{hf}", bufs=4
                    )
                    nc.vector.reciprocal_approx_fast(rcp[:], ps[:])
                    nc.vector.tensor_mul(dst[hf][:], numer[hf][:], rcp[:])

            for it_ in range(N_ITERS):
                for s in range(S_PER_CORE):
                    half_step(s, "Bt", "ct", 2 * it_)
                for s in range(S_PER_CORE):
                    half_step(s, "Ut", "Rt", 2 * it_ + 1)

            for s in range(S_PER_CORE):
                half_step(s, "Bt", "ct", 99)  # Bt = Vt
            for s in range(S_PER_CORE):
                st = sink_state[s]
                KM_t, Bt, Ut = st["KM_t"], st["Bt"], st["Ut"]
                for hf in range(2):
                    ps = ps_mm.tile([128, L], f32, tag="hdn", name=f"pt_{s}_{hf}")
                    nc.tensor.matmul(
                        ps[:], KM_t[0][:, 128 * hf : 128 * (hf + 1)], Bt[0][:],
                        start=True, stop=False,
                    )
                    nc.tensor.matmul(
                        ps[:], KM_t[1][:, 128 * hf : 128 * (hf + 1)], Bt[1][:],
                        start=False, stop=True,
                    )
                    junk = small_pool.tile([128, L], bf16, tag="junk", name=f"junk_{s}_{hf}")
                    nc.vector.scalar_tensor_tensor(
                        out=junk[:],
                        in0=Ut[hf][:],
                        scalar=1.0,
                        in1=ps[:],
                        op0=ALU.mult,
                        op1=ALU.mult,
                        accum_out=lossc[:, 2 * s + hf : 2 * s + hf + 1],
                    )

            # ---- total wloss ----
            wl_ps = ps_s2.tile([1, 2 * S_PER_CORE], f32, tag="s2", name="wl_ps")
            nc.tensor.matmul(wl_ps[:], ones_f32[:], lossc[:])
            wl_sb = const_pool.tile([1, 1], f32)
            nc.vector.tensor_reduce(wl_sb[:], wl_ps[:], axis=AX.X, op=ALU.add)
            wl_out_sb = const_pool.tile([1, 1], f32)
            nc.vector.tensor_copy(wl_out_sb[:], wl_sb[:])
            nc.sync.dma_start(out=wloss_out[:, :], in_=wl_out_sb[:])

    nc.finalize()
    return nc


_NC_CACHE = None


def _get_nc():
    global _NC_CACHE
    if _NC_CACHE is None:
        _NC_CACHE = build_nc()
    return _NC_CACHE


LAST_EXEC_NS = None


def kernel(f, head, W1, b1, W2, b2):
    f = np.ascontiguousarray(f, dtype=np.float32)
    head = np.ascontiguousarray(head, dtype=np.float32)
    W1 = np.ascontiguousarray(W1, dtype=np.float32)
    b1 = np.ascontiguousarray(b1, dtype=np.float32)
    W2 = np.ascontiguousarray(W2, dtype=np.float32)
    b2 = np.ascontiguousarray(b2, dtype=np.float32)

    from concourse.bass_utils import run_bass_kernel_spmd

    nc = _get_nc()
    in_maps = []
    for cid in range(N_CORES):
        s0 = cid * S_PER_CORE
        in_maps.append(
            {
                "f": f[s0 : s0 + S_PER_CORE],
                "head": head[s0 : s0 + S_PER_CORE],
                "W1": W1,
                "b1": b1,
                "W2": W2,
                "b2": b2,
            }
        )
    trace = bool(int(os.environ.get("ARC_KERNEL_TRACE", "0")))
    res = run_bass_kernel_spmd(nc, in_maps, list(range(N_CORES)), trace=trace)
    if trace:
        global LAST_EXEC_NS
        LAST_EXEC_NS = res.exec_time_ns
    scores = np.concatenate(
        [np.asarray(r["scores"]).reshape(S_PER_CORE, L, L) for r in res.results],
        axis=0,
    )
    wloss = np.float32(
        sum(float(np.asarray(r["wloss"]).reshape(-1)[0]) for r in res.results)
    )
    return scores, np.asarray(wloss, dtype=np.float32)


# revision 31
# speedup vs baseline: 1.7268x; 1.0656x over previous
"""Trainium2 Bass kernel for nn_ArcPredictorWloss.

Reference computation (per sample s of n=16, l=256, h=hid=128):
  scores = tanh(f.reshape(l*l, h) @ W1 + b1) @ W2 + b2          # (l, l)
  C[i,j] = sum_h f[i,j,h] * f[j,i,h]   (symmetric)
  Cn = C / (max|C| + eps)
  r = softmax(scores + eps, axis=-1);  c = (head+eps)/sum(head+eps, -1)
  per-row Sinkhorn (20 iters) with K = exp(-20*Cn), KM = K*Cn
  wloss = sum over rows/samples of u . (KM v)

Sharding: data-parallel over n across 8 cores (2 samples per core).
All heavy compute in bf16 (validated: scores err ~3.5e-3 rel-to-max,
wloss err ~1.2e-3 worst-case vs fp64), fp32 PSUM accumulation.

Layout strategy per sample (l=256 -> 2x2 grid of 128x128 blocks):
  - A(r,c)[p,jj,h] = f[128r+p, 128c+jj, h]   (natural, row-partition)
  - B(r,c)[p,jj,h] = f[128c+jj, 128r+p, h]   (partner, row-partition)
    Both loaded straight from HBM by casting gpsimd DMAs (fp32->bf16).
    Loaded: A00,B00,A01,B01,A11,B11 - block10's bytes arrive as B01, so
    off-diagonal data is read once; diagonal blocks are read twice.
  - C(r,c) = sum_h A(r,c)*B(r,c): multiply on GPSIMD (otherwise idle),
    reduce on DVE.  C(1,0) = C(0,1)^T via PE transpose.
  - MLP: PE-transpose of [128 rows, h] col-tiles -> Xt, stage-1 vs W1,
    tanh on ACT (PSUM->SBUF), stage-2 as [arcs,1] columns (lhsT=hdn
    tile, rhs=W2) accumulated into [128,32] PSUM chunks -> scores in
    natural layout (block10 via B01 lands transposed; one extra PE
    transpose fixes it up).
  - softmax over free dim; Sinkhorn state transposed [bin, row]; K/KM
    symmetric so the loop is matmul + approx-reciprocal + multiply.
    The +EPS inside the loop is a rank-1 (eps-row x ones-row)
    PSUM-accumulated matmul.  Both samples' Sinkhorn chains are emitted
    interleaved so the two dependency chains overlap on the engines.
"""

import os
import sys
import numpy as np

sys.path.insert(0, "/opt/trn_rl_repo")

LAM = 20.0
N_ITERS = 20
EPS = 1e-8

N, L, H = 16, 256, 128
N_CORES = 8
S_PER_CORE = N // N_CORES  # 2


def build_nc():
    import concourse.bass as bass
    import concourse.mybir as mybir
    from concourse import bacc, masks
    from concourse.tile import TileContext

    f32 = mybir.dt.float32
    bf16 = mybir.dt.bfloat16
    AF = mybir.ActivationFunctionType
    ALU = mybir.AluOpType
    AX = mybir.AxisListType

    nc = bacc.Bacc("TRN2", target_bir_lowering=False)

    f_in = nc.declare_dram_parameter("f", [S_PER_CORE, L, L, H], f32, isOutput=False)
    head_in = nc.declare_dram_parameter("head", [S_PER_CORE, L, L], f32, isOutput=False)
    w1_in = nc.declare_dram_parameter("W1", [H, H], f32, isOutput=False)
    b1_in = nc.declare_dram_parameter("b1", [H], f32, isOutput=False)
    w2_in = nc.declare_dram_parameter("W2", [H, 1], f32, isOutput=False)
    b2_in = nc.declare_dram_parameter("b2", [1], f32, isOutput=False)
    scores_out = nc.declare_dram_parameter(
        "scores", [S_PER_CORE, L, L], f32, isOutput=True
    )
    wloss_out = nc.declare_dram_parameter("wloss", [1, 1], f32, isOutput=True)

    with TileContext(nc) as tc:
        from contextlib import ExitStack

        ctx = ExitStack()
        with ctx:
            const_pool = ctx.enter_context(tc.tile_pool(name="const", bufs=1))
            blk_pool = ctx.enter_context(tc.tile_pool(name="blk", bufs=4))
            xt_pool = ctx.enter_context(tc.tile_pool(name="xt", bufs=5))
            hdn_pool = ctx.enter_context(tc.tile_pool(name="hdn", bufs=5))
            prod_pool = ctx.enter_context(tc.tile_pool(name="prod", bufs=3))
            small_pool = ctx.enter_context(tc.tile_pool(name="small", bufs=2))
            samp_pool = ctx.enter_context(tc.tile_pool(name="samp", bufs=2))
            ps_xt = ctx.enter_context(tc.tile_pool(name="ps_xt", bufs=3, space="PSUM"))
            ps_mm = ctx.enter_context(tc.tile_pool(name="ps_mm", bufs=3, space="PSUM"))
            ps_s2 = ctx.enter_context(tc.tile_pool(name="ps_s2", bufs=2, space="PSUM"))

            # ---- constants / weights ----
            ident_bf = const_pool.tile([128, 128], bf16)
            masks.make_identity(nc, ident_bf[:])
            ident_f32 = const_pool.tile([128, 128], f32)
            masks.make_identity(nc, ident_f32[:])
            ones_f32 = const_pool.tile([128, 1], f32)
            nc.vector.memset(ones_f32[:], 1.0)
            ones_row_f32 = const_pool.tile([1, 128], f32)
            nc.vector.memset(ones_row_f32[:], 1.0)
            eps_row_bf = const_pool.tile([1, 128], bf16)
            nc.vector.memset(eps_row_bf[:], EPS)
            ones_row_bf = const_pool.tile([1, L], bf16)
            nc.vector.memset(ones_row_bf[:], 1.0)

            w1_f32 = const_pool.tile([H, H], f32)
            nc.sync.dma_start(out=w1_f32[:], in_=w1_in[:, :])
            w1_bf = const_pool.tile([H, H], bf16)
            nc.vector.tensor_copy(w1_bf[:], w1_f32[:])

            b1_sb = const_pool.tile([H, 1], f32)
            nc.sync.dma_start(
                out=b1_sb[:], in_=b1_in[:].rearrange("(h one) -> h one", one=1)
            )

            w2_f32 = const_pool.tile([H, 1], f32)
            nc.sync.dma_start(out=w2_f32[:], in_=w2_in[:, :])
            w2_bf = const_pool.tile([H, 1], bf16)
            nc.vector.tensor_copy(w2_bf[:], w2_f32[:])

            b2_sb = const_pool.tile([1, 1], f32)
            nc.sync.dma_start(
                out=b2_sb[:], in_=b2_in[:].rearrange("(o one) -> o one", one=1)
            )
            b2_bcast = const_pool.tile([128, 1], f32)
            nc.sync.dma_start(
                out=b2_bcast[:],
                in_=b2_in[:]
                .rearrange("(o one) -> o one", one=1)
                .to_broadcast((128, 1)),
            )

            # per-(sample,half) loss partials
            lossc = const_pool.tile([128, 2 * S_PER_CORE], f32)
            sink_state = []

            for s in range(S_PER_CORE):
                # ============ head -> ct (transposed target hist) ==========
                head_nat = [
                    samp_pool.tile([128, L], f32, tag="head", name=f"head_{s}_{i}")
                    for i in range(2)
                ]
                c_nat = [
                    samp_pool.tile([128, L], bf16, tag="cnat", name=f"cnat_{s}_{i}")
                    for i in range(2)
                ]
                for t in range(2):
                    nc.sync.dma_start(
                        out=head_nat[t][:], in_=head_in[s, 128 * t : 128 * (t + 1), :]
                    )
                    rs = small_pool.tile([128, 1], f32, tag="rs", name=f"rs_{s}_{t}")
                    nc.vector.tensor_reduce(rs[:], head_nat[t][:], axis=AX.X, op=ALU.add)
                    rs_eps = small_pool.tile(
                        [128, 1], f32, tag="rs_eps", name=f"rse_{s}_{t}"
                    )
                    nc.vector.tensor_scalar_add(rs_eps[:], rs[:], float(L) * EPS)
                    rec = small_pool.tile([128, 1], f32, tag="rec", name=f"rec_{s}_{t}")
                    nc.vector.reciprocal(rec[:], rs_eps[:])
                    # c = (head + eps) * (1/sum) in one DVE pass
                    nc.vector.tensor_scalar(
                        out=c_nat[t][:],
                        in0=head_nat[t][:],
                        scalar1=EPS,
                        scalar2=rec[:],
                        op0=ALU.add,
                        op1=ALU.mult,
                    )
                ct = [
                    samp_pool.tile([128, L], bf16, tag="ct", bufs=4, name=f"ct_{s}_{i}")
                    for i in range(2)
                ]
                for jt in range(2):
                    for it in range(2):
                        tp = ps_xt.tile(
                            [128, 128], bf16, tag="xt", name=f"tp_{s}_{jt}_{it}"
                        )
                        nc.tensor.transpose(
                            tp[:], c_nat[it][:, 128 * jt : 128 * (jt + 1)], ident_bf[:]
                        )
                        nc.vector.tensor_copy(ct[jt][:, 128 * it : 128 * (it + 1)], tp[:])

                # ============ stream f: MLP + C ============================
                scores_nat = [
                    samp_pool.tile([128, L], f32, tag="snat", name=f"snat_{s}_{i}")
                    for i in range(2)
                ]
                C_t = [
                    samp_pool.tile([128, L], bf16, tag="C", name=f"C_{s}_{i}")
                    for i in range(2)
                ]

                def load_A(r, c):
                    # split in col-halves so downstream MLP groups can start
                    # as soon as the first 4 MiB lands
                    t = blk_pool.tile(
                        [128, 128, H], bf16, tag="blk", name=f"A_{s}_{r}_{c}"
                    )
                    for jh in range(2):
                        nc.gpsimd.dma_start(
                            out=t[:, 64 * jh : 64 * (jh + 1), :],
                            in_=f_in[
                                s,
                                128 * r : 128 * (r + 1),
                                128 * c + 64 * jh : 128 * c + 64 * (jh + 1),
                                :,
                            ],
                        )
                    return t

                def load_B(r, c):
                    # B[p, jj, h] = f[128c+jj, 128r+p, h] (strided direct
                    # load; split in jj-halves to stay under the 16384
                    # descriptor-per-DMA limit)
                    t = blk_pool.tile(
                        [128, 128, H], bf16, tag="blk", name=f"B_{s}_{r}_{c}"
                    )
                    for jh in range(2):
                        src = f_in[
                            s,
                            128 * c + 64 * jh : 128 * c + 64 * (jh + 1),
                            128 * r : 128 * (r + 1),
                            :,
                        ].rearrange("j p h -> p j h")
                        nc.gpsimd.dma_start(
                            out=t[:, 64 * jh : 64 * (jh + 1), :], in_=src
                        )
                    return t

                def shuffle_B_diag(src_tile, r):
                    # diagonal partner layout from the already-loaded A(r,r):
                    # per-row SBUF->SBUF DMAs on the otherwise-idle HWDGE,
                    # saving the 8 MiB HBM re-read of the block
                    t = blk_pool.tile(
                        [128, 128, H], bf16, tag="blk", name=f"B_{s}_{r}_{r}"
                    )
                    for jj in range(128):
                        nc.sync.dma_start(
                            out=t[:, jj, :], in_=src_tile[jj : jj + 1, :, :]
                        )
                    return t

                def mlp_block(blk, rt, cl, transposed_out=None):
                    # blk[:, jj, :] = 128 arcs; for A(r,c) these are
                    # (row 128r+p, col 128c+jj) -> scores_nat[rt] col chunks.
                    # For B(0,1) (= block10 data) they are (row 128+jj, col p)
                    # -> transposed staging, fixed up by one PE transpose.
                    s2_ps = None
                    for g in range(32):
                        xt_ps = ps_xt.tile(
                            [128, 512], bf16, tag="xt", name=f"xtps_{s}_{rt}_{cl}_{g}"
                        )
                        for t in range(4):
                            jj = 4 * g + t
                            nc.tensor.transpose(
                                xt_ps[:, 128 * t : 128 * (t + 1)],
                                blk[:, jj, :],
                                ident_bf[:],
                            )
                        xt_sb = xt_pool.tile(
                            [128, 512], bf16, tag="xt_sb", name=f"xtsb_{s}_{rt}_{cl}_{g}"
                        )
                        if g % 2 == 0:
                            nc.vector.tensor_copy(xt_sb[:], xt_ps[:])
                        else:
                            nc.scalar.copy(xt_sb[:], xt_ps[:])
                        hdn_ps = ps_mm.tile(
                            [128, 512], f32, tag="hdn", name=f"hdnps_{s}_{rt}_{cl}_{g}"
                        )
                        nc.tensor.matmul(hdn_ps[:], w1_bf[:], xt_sb[:])
                        hdn_sb = hdn_pool.tile(
                            [128, 512], bf16, tag="hdn_sb",
                            name=f"hdnsb_{s}_{rt}_{cl}_{g}",
                        )
                        nc.scalar.activation(hdn_sb[:], hdn_ps[:], AF.Tanh, bias=b1_sb[:])
                        if g % 8 == 0:
                            s2_ps = ps_s2.tile(
                                [128, 32], f32, tag="s2", name=f"s2ps_{s}_{rt}_{cl}_{g}"
                            )
                        for t in range(4):
                            jj = 4 * g + t
                            nc.tensor.matmul(
                                s2_ps[:, jj % 32 : jj % 32 + 1],
                                hdn_sb[:, 128 * t : 128 * (t + 1)],
                                w2_bf[:],
                            )
                        if g % 8 == 7:
                            q = g // 8
                            if transposed_out is None:
                                nc.vector.tensor_scalar_add(
                                    scores_nat[rt][
                                        :, 128 * cl + 32 * q : 128 * cl + 32 * (q + 1)
                                    ],
                                    s2_ps[:],
                                    b2_bcast[:],
                                )
                            else:
                                nc.vector.tensor_copy(
                                    transposed_out[:, 32 * q : 32 * (q + 1)], s2_ps[:]
                                )

                def c_compute(rt, col0, A, B):
                    for q in range(8):
                        sl = slice(16 * q, 16 * (q + 1))
                        prod = prod_pool.tile(
                            [128, 16, H], bf16, tag="prod",
                            name=f"prod_{s}_{rt}_{col0}_{q}",
                        )
                        nc.vector.tensor_mul(prod[:], A[:, sl, :], B[:, sl, :])
                        with nc.allow_low_precision(
                            "C in bf16 validated: wloss err ~1e-3"
                        ):
                            nc.vector.tensor_reduce(
                                C_t[rt][:, col0 + 16 * q : col0 + 16 * (q + 1)],
                                prod[:],
                                axis=AX.X,
                                op=ALU.add,
                            )

                # ---- (0,0) ----
                A00 = load_A(0, 0)
                B00 = load_B(0, 0)
                mlp_block(A00, 0, 0)
                c_compute(0, 0, A00, B00)
                # ---- (0,1) + (1,0) ----
                A01 = load_A(0, 1)
                B01 = load_B(0, 1)
                mlp_block(A01, 0, 1)
                sT10 = samp_pool.tile([128, 128], f32, tag="sT10", name=f"sT10_{s}")
                mlp_block(B01, 1, 0, transposed_out=sT10)
                tpS = ps_xt.tile([128, 128], f32, tag="xt", name=f"tpS_{s}")
                nc.tensor.transpose(tpS[:], sT10[:], ident_f32[:])
                nc.vector.tensor_scalar_add(scores_nat[1][:, 0:128], tpS[:], b2_bcast[:])
                c_compute(0, 128, A01, B01)
                # C(1,0) = C(0,1)^T
                tp_c = ps_xt.tile([128, 128], bf16, tag="xt", name=f"tpc_{s}")
                nc.tensor.transpose(tp_c[:], C_t[0][:, 128:256], ident_bf[:])
                nc.vector.tensor_copy(C_t[1][:, 0:128], tp_c[:])
                # ---- (1,1) ----
                A11 = load_A(1, 1)
                B11 = load_B(1, 1)
                mlp_block(A11, 1, 1)
                c_compute(1, 128, A11, B11)

                # ============ scores output ================================
                for it in range(2):
                    nc.sync.dma_start(
                        out=scores_out[s, 128 * it : 128 * (it + 1), :],
                        in_=scores_nat[it][:],
                    )

                # ============ softmax (natural) + transpose to Rt ==========
                R_nat = [
                    samp_pool.tile([128, L], bf16, tag="Rnat", name=f"Rnat_{s}_{i}")
                    for i in range(2)
                ]
                for it in range(2):
                    mxr = small_pool.tile([128, 1], f32, tag="mxr", name=f"mxr_{s}_{it}")
                    nc.vector.tensor_reduce(
                        mxr[:], scores_nat[it][:], axis=AX.X, op=ALU.max
                    )
                    nmxr = small_pool.tile([128, 1], f32, tag="nmxr", name=f"nmxr_{s}_{it}")
                    nc.vector.tensor_scalar_mul(nmxr[:], mxr[:], -1.0)
                    e_nat = small_pool.tile([128, L], f32, tag="enat", name=f"enat_{s}_{it}")
                    zs = small_pool.tile([128, 1], f32, tag="zs", name=f"zs_{s}_{it}")
                    nc.scalar.activation(
                        e_nat[:], scores_nat[it][:], AF.Exp, bias=nmxr[:], accum_out=zs[:]
                    )
                    zrec = small_pool.tile([128, 1], f32, tag="zrec", name=f"zrec_{s}_{it}")
                    nc.vector.reciprocal(zrec[:], zs[:])
                    nc.vector.tensor_scalar_mul(R_nat[it][:], e_nat[:], zrec[:])
                Rt = [
                    samp_pool.tile([128, L], bf16, tag="Rt", bufs=4, name=f"Rt_{s}_{i}")
                    for i in range(2)
                ]
                for jt in range(2):
                    for it in range(2):
                        tpr = ps_xt.tile(
                            [128, 128], bf16, tag="xt", name=f"tpr_{s}_{jt}_{it}"
                        )
                        nc.tensor.transpose(
                            tpr[:], R_nat[it][:, 128 * jt : 128 * (jt + 1)], ident_bf[:]
                        )
                        nc.vector.tensor_copy(Rt[jt][:, 128 * it : 128 * (it + 1)], tpr[:])

                # ============ K, KM ========================================
                mx = small_pool.tile([128, 1], f32, tag="mx", name=f"mx_{s}")
                mx2 = small_pool.tile([128, 1], f32, tag="mx2", name=f"mx2_{s}")
                nc.vector.tensor_reduce(
                    mx[:], C_t[0][:], axis=AX.X, op=ALU.max, apply_absolute_value=True
                )
                nc.vector.tensor_reduce(
                    mx2[:], C_t[1][:], axis=AX.X, op=ALU.max, apply_absolute_value=True
                )
                mxc = small_pool.tile([128, 1], f32, tag="mxc", name=f"mxc_{s}")
                nc.vector.tensor_max(mxc[:], mx[:], mx2[:])
                mxt = ps_xt.tile([1, 128], f32, tag="xt", name=f"mxt_{s}")
                nc.tensor.transpose(mxt[:], mxc[:], ident_f32[:])
                mxs = small_pool.tile([1, 1], f32, tag="mxs", name=f"mxs_{s}")
                nc.vector.tensor_reduce(mxs[:], mxt[:], axis=AX.X, op=ALU.max)
                mxe = small_pool.tile([1, 1], f32, tag="mxe", name=f"mxe_{s}")
                nc.vector.tensor_scalar_add(mxe[:], mxs[:], EPS)
                inv = small_pool.tile([1, 1], f32, tag="inv", name=f"inv_{s}")
                nc.vector.reciprocal(inv[:], mxe[:])
                sk = small_pool.tile([1, 1], f32, tag="sk", name=f"sk_{s}")
                nc.scalar.mul(sk[:], inv[:], -LAM)
                inv_b = small_pool.tile([128, 1], f32, tag="inv_b", name=f"invb_{s}")
                inv_ps = ps_xt.tile([128, 1], f32, tag="xt", name=f"invps_{s}")
                nc.tensor.matmul(inv_ps[:], ones_row_f32[:], inv[:])
                nc.vector.tensor_copy(inv_b[:], inv_ps[:])
                sk_b = small_pool.tile([128, 1], f32, tag="sk_b", name=f"skb_{s}")
                sk_ps = ps_xt.tile([128, 1], f32, tag="xt", name=f"skps2_{s}")
                nc.tensor.matmul(sk_ps[:], ones_row_f32[:], sk[:])
                nc.vector.tensor_copy(sk_b[:], sk_ps[:])

                K_t = [
                    samp_pool.tile([128, L], bf16, tag="K", bufs=4, name=f"K_{s}_{i}")
                    for i in range(2)
                ]
                KM_t = [
                    samp_pool.tile([128, L], bf16, tag="KM", bufs=4, name=f"KM_{s}_{i}")
                    for i in range(2)
                ]
                for t in range(2):
                    nc.scalar.activation(K_t[t][:], C_t[t][:], AF.Exp, scale=sk_b[:])
                    nc.vector.scalar_tensor_tensor(
                        out=KM_t[t][:],
                        in0=K_t[t][:],
                        scalar=inv_b[:],
                        in1=C_t[t][:],
                        op0=ALU.mult,
                        op1=ALU.mult,
                    )

                # ============ Sinkhorn state (loop runs after both samples) =
                Ut = [
                    samp_pool.tile([128, L], bf16, tag="Ut", bufs=4, name=f"Ut_{s}_{i}")
                    for i in range(2)
                ]
                for t in range(2):
                    nc.vector.memset(Ut[t][:], 1.0 / L)
                Bt = [
                    samp_pool.tile([128, L], bf16, tag="Bt", bufs=4, name=f"Bt_{s}_{i}")
                    for i in range(2)
                ]
                sink_state.append(dict(ct=ct, Rt=Rt, K_t=K_t, KM_t=KM_t, Ut=Ut, Bt=Bt))

            # ============ Sinkhorn: both samples interleaved ================
            def half_step(s, dst_name, numer_name, it_tag):
                st = sink_state[s]
                dst, numer = st[dst_name], st[numer_name]
                rhs_tiles = st["Ut"] if dst_name == "Bt" else st["Bt"]
                lhs_tiles = st["K_t"]
                for hf in range(2):
                    ps = ps_mm.tile(
                        [128, L], f32, tag="hdn", name=f"skps_{s}_{it_tag}_{hf}"
                    )
                    nc.tensor.matmul(
                        ps[:], eps_row_bf[:], ones_row_bf[:], start=True, stop=False
                    )
                    nc.tensor.matmul(
                        ps[:],
                        lhs_tiles[0][:, 128 * hf : 128 * (hf + 1)],
                        rhs_tiles[0][:],
                        start=False,
                        stop=False,
                    )
                    nc.tensor.matmul(
                        ps[:],
                        lhs_tiles[1][:, 128 * hf : 128 * (hf + 1)],
                        rhs_tiles[1][:],
                        start=False,
                        stop=True,
                    )
                    rcp = small_pool.tile(
                        [128, L], f32, tag="rcp", name=f"rcp_{s}_{it_tag}_{hf}", bufs=4
                    )
                    nc.vector.reciprocal_approx_fast(rcp[:], ps[:])
                    nc.vector.tensor_mul(dst[hf][:], numer[hf][:], rcp[:])

            for it_ in range(N_ITERS):
                for s in range(S_PER_CORE):
                    half_step(s, "Bt", "ct", 2 * it_)
                for s in range(S_PER_CORE):
                    half_step(s, "Ut", "Rt", 2 * it_ + 1)

            for s in range(S_PER_CORE):
                half_step(s, "Bt", "ct", 99)  # Bt = Vt
            for s in range(S_PER_CORE):
                st = sink_state[s]
                KM_t, Bt, Ut = st["KM_t"], st["Bt"], st["Ut"]
                for hf in range(2):
                    ps = ps_mm.tile([128, L], f32, tag="hdn", name=f"pt_{s}_{hf}")
                    nc.tensor.matmul(
                        ps[:], KM_t[0][:, 128 * hf : 128 * (hf + 1)], Bt[0][:],
                        start=True, stop=False,
                    )
                    nc.tensor.matmul(
                        ps[:], KM_t[1][:, 128 * hf : 128 * (hf + 1)], Bt[1][:],
                        start=False, stop=True,
                    )
                    junk = small_pool.tile([128, L], bf16, tag="junk", name=f"junk_{s}_{hf}")
                    nc.vector.scalar_tensor_tensor(
                        out=junk[:],
                        in0=Ut[hf][:],
                        scalar=1.0,
                        in1=ps[:],
                        op0=ALU.mult,
                        op1=ALU.mult,
                        accum_out=lossc[:, 2 * s + hf : 2 * s + hf + 1],
                    )

            # ---- total wloss ----
            wl_ps = ps_s2.tile([1, 2 * S_PER_CORE], f32, tag="s2", name="wl_ps")
            nc.tensor.matmul(wl_ps[:], ones_f32[:], lossc[:])
            wl_sb = const_pool.tile([1, 1], f32)
            nc.vector.tensor_reduce(wl_sb[:], wl_ps[:], axis=AX.X, op=ALU.add)
            wl_out_sb = const_pool.tile([1, 1], f32)
            nc.vector.tensor_copy(wl_out_sb[:], wl_sb[:])
            nc.sync.dma_start(out=wloss_out[:, :], in_=wl_out_sb[:])

    nc.finalize()
    return nc


_NC_CACHE = None


def _get_nc():
    global _NC_CACHE
    if _NC_CACHE is None:
        _NC_CACHE = build_nc()
    return _NC_CACHE


LAST_EXEC_NS = None


def kernel(f, head, W1, b1, W2, b2):
    f = np.ascontiguousarray(f, dtype=np.float32)
    head = np.ascontiguousarray(head, dtype=np.float32)
    W1 = np.ascontiguousarray(W1, dtype=np.float32)
    b1 = np.ascontiguousarray(b1, dtype=np.float32)
    W2 = np.ascontiguousarray(W2, dtype=np.float32)
    b2 = np.ascontiguousarray(b2, dtype=np.float32)

    from concourse.bass_utils import run_bass_kernel_spmd

    nc = _get_nc()
    in_maps = []
    for cid in range(N_CORES):
        s0 = cid * S_PER_CORE
        in_maps.append(
            {
                "f": f[s0 : s0 + S_PER_CORE],
                "head": head[s0 : s0 + S_PER_CORE],
                "W1": W1,
                "b1": b1,
                "W2": W2,
                "b2": b2,
            }
        )
    trace = bool(int(os.environ.get("ARC_KERNEL_TRACE", "0")))
    res = run_bass_kernel_spmd(nc, in_maps, list(range(N_CORES)), trace=trace)
    if trace:
        global LAST_EXEC_NS
        LAST_EXEC_NS = res.exec_time_ns
    scores = np.concatenate(
        [np.asarray(r["scores"]).reshape(S_PER_CORE, L, L) for r in res.results],
        axis=0,
    )
    wloss = np.float32(
        sum(float(np.asarray(r["wloss"]).reshape(-1)[0]) for r in res.results)
    )
    return scores, np.asarray(wloss, dtype=np.float32)


# revision 32
# speedup vs baseline: 1.7607x; 1.0197x over previous
"""Trainium2 Bass kernel for nn_ArcPredictorWloss.

Reference computation (per sample s of n=16, l=256, h=hid=128):
  scores = tanh(f.reshape(l*l, h) @ W1 + b1) @ W2 + b2          # (l, l)
  C[i,j] = sum_h f[i,j,h] * f[j,i,h]   (symmetric)
  Cn = C / (max|C| + eps)
  r = softmax(scores + eps, axis=-1);  c = (head+eps)/sum(head+eps, -1)
  per-row Sinkhorn (20 iters) with K = exp(-20*Cn), KM = K*Cn
  wloss = sum over rows/samples of u . (KM v)

Sharding: data-parallel over n across 8 cores (2 samples per core).
All heavy compute in bf16 (validated: scores err ~3.5e-3 rel-to-max,
wloss err ~1.2e-3 worst-case vs fp64), fp32 PSUM accumulation.

Layout strategy per sample (l=256 -> 2x2 grid of 128x128 blocks):
  - A(r,c)[p,jj,h] = f[128r+p, 128c+jj, h]   (natural, row-partition)
  - B(r,c)[p,jj,h] = f[128c+jj, 128r+p, h]   (partner, row-partition)
    Both loaded straight from HBM by casting gpsimd DMAs (fp32->bf16).
    Loaded: A00,B00,A01,B01,A11,B11 - block10's bytes arrive as B01, so
    off-diagonal data is read once; diagonal blocks are read twice.
  - C(r,c) = sum_h A(r,c)*B(r,c): multiply on GPSIMD (otherwise idle),
    reduce on DVE.  C(1,0) = C(0,1)^T via PE transpose.
  - MLP: PE-transpose of [128 rows, h] col-tiles -> Xt, stage-1 vs W1,
    tanh on ACT (PSUM->SBUF), stage-2 as [arcs,1] columns (lhsT=hdn
    tile, rhs=W2) accumulated into [128,32] PSUM chunks -> scores in
    natural layout (block10 via B01 lands transposed; one extra PE
    transpose fixes it up).
  - softmax over free dim; Sinkhorn state transposed [bin, row]; K/KM
    symmetric so the loop is matmul + approx-reciprocal + multiply.
    The +EPS inside the loop is a rank-1 (eps-row x ones-row)
    PSUM-accumulated matmul.  Both samples' Sinkhorn chains are emitted
    interleaved so the two dependency chains overlap on the engines.
"""

import os
import sys
import numpy as np

sys.path.insert(0, "/opt/trn_rl_repo")

LAM = 20.0
N_ITERS = 20
EPS = 1e-8

N, L, H = 16, 256, 128
N_CORES = 8
S_PER_CORE = N // N_CORES  # 2


def build_nc():
    import concourse.bass as bass
    import concourse.mybir as mybir
    from concourse import bacc, masks
    from concourse.tile import TileContext

    f32 = mybir.dt.float32
    bf16 = mybir.dt.bfloat16
    AF = mybir.ActivationFunctionType
    ALU = mybir.AluOpType
    AX = mybir.AxisListType

    nc = bacc.Bacc("TRN2", target_bir_lowering=False)

    f_in = nc.declare_dram_parameter("f", [S_PER_CORE, L, L, H], f32, isOutput=False)
    head_in = nc.declare_dram_parameter("head", [S_PER_CORE, L, L], f32, isOutput=False)
    w1_in = nc.declare_dram_parameter("W1", [H, H], f32, isOutput=False)
    b1_in = nc.declare_dram_parameter("b1", [H], f32, isOutput=False)
    w2_in = nc.declare_dram_parameter("W2", [H, 1], f32, isOutput=False)
    b2_in = nc.declare_dram_parameter("b2", [1], f32, isOutput=False)
    scores_out = nc.declare_dram_parameter(
        "scores", [S_PER_CORE, L, L], f32, isOutput=True
    )
    wloss_out = nc.declare_dram_parameter("wloss", [1, 1], f32, isOutput=True)

    with TileContext(nc) as tc:
        from contextlib import ExitStack

        ctx = ExitStack()
        with ctx:
            const_pool = ctx.enter_context(tc.tile_pool(name="const", bufs=1))
            blk_pool = ctx.enter_context(tc.tile_pool(name="blk", bufs=4))
            xt_pool = ctx.enter_context(tc.tile_pool(name="xt", bufs=5))
            hdn_pool = ctx.enter_context(tc.tile_pool(name="hdn", bufs=5))
            prod_pool = ctx.enter_context(tc.tile_pool(name="prod", bufs=3))
            small_pool = ctx.enter_context(tc.tile_pool(name="small", bufs=2))
            samp_pool = ctx.enter_context(tc.tile_pool(name="samp", bufs=2))
            ps_xt = ctx.enter_context(tc.tile_pool(name="ps_xt", bufs=3, space="PSUM"))
            ps_mm = ctx.enter_context(tc.tile_pool(name="ps_mm", bufs=3, space="PSUM"))
            ps_s2 = ctx.enter_context(tc.tile_pool(name="ps_s2", bufs=2, space="PSUM"))

            # ---- constants / weights ----
            ident_bf = const_pool.tile([128, 128], bf16)
            masks.make_identity(nc, ident_bf[:])
            ident_f32 = const_pool.tile([128, 128], f32)
            masks.make_identity(nc, ident_f32[:])
            ones_f32 = const_pool.tile([128, 1], f32)
            nc.vector.memset(ones_f32[:], 1.0)
            ones_row_f32 = const_pool.tile([1, 128], f32)
            nc.vector.memset(ones_row_f32[:], 1.0)
            eps_row_bf = const_pool.tile([1, 128], bf16)
            nc.vector.memset(eps_row_bf[:], EPS)
            ones_row_bf = const_pool.tile([1, L], bf16)
            nc.vector.memset(ones_row_bf[:], 1.0)

            w1_f32 = const_pool.tile([H, H], f32)
            nc.sync.dma_start(out=w1_f32[:], in_=w1_in[:, :])
            w1_bf = const_pool.tile([H, H], bf16)
            nc.vector.tensor_copy(w1_bf[:], w1_f32[:])

            b1_sb = const_pool.tile([H, 1], f32)
            nc.sync.dma_start(
                out=b1_sb[:], in_=b1_in[:].rearrange("(h one) -> h one", one=1)
            )

            w2_f32 = const_pool.tile([H, 1], f32)
            nc.sync.dma_start(out=w2_f32[:], in_=w2_in[:, :])
            w2_bf = const_pool.tile([H, 1], bf16)
            nc.vector.tensor_copy(w2_bf[:], w2_f32[:])

            b2_sb = const_pool.tile([1, 1], f32)
            nc.sync.dma_start(
                out=b2_sb[:], in_=b2_in[:].rearrange("(o one) -> o one", one=1)
            )
            b2_bcast = const_pool.tile([128, 1], f32)
            nc.sync.dma_start(
                out=b2_bcast[:],
                in_=b2_in[:]
                .rearrange("(o one) -> o one", one=1)
                .to_broadcast((128, 1)),
            )

            # per-(sample,half) loss partials
            lossc = const_pool.tile([128, 2 * S_PER_CORE], f32)
            sink_state = []

            for s in range(S_PER_CORE):
                # ============ head -> ct (transposed target hist) ==========
                head_nat = [
                    samp_pool.tile([128, L], f32, tag="head", name=f"head_{s}_{i}")
                    for i in range(2)
                ]
                c_nat = [
                    samp_pool.tile([128, L], bf16, tag="cnat", name=f"cnat_{s}_{i}")
                    for i in range(2)
                ]
                for t in range(2):
                    nc.sync.dma_start(
                        out=head_nat[t][:], in_=head_in[s, 128 * t : 128 * (t + 1), :]
                    )
                    rs = small_pool.tile([128, 1], f32, tag="rs", name=f"rs_{s}_{t}")
                    nc.vector.tensor_reduce(rs[:], head_nat[t][:], axis=AX.X, op=ALU.add)
                    rs_eps = small_pool.tile(
                        [128, 1], f32, tag="rs_eps", name=f"rse_{s}_{t}"
                    )
                    nc.vector.tensor_scalar_add(rs_eps[:], rs[:], float(L) * EPS)
                    rec = small_pool.tile([128, 1], f32, tag="rec", name=f"rec_{s}_{t}")
                    nc.vector.reciprocal(rec[:], rs_eps[:])
                    # c = (head + eps) * (1/sum) in one DVE pass
                    nc.vector.tensor_scalar(
                        out=c_nat[t][:],
                        in0=head_nat[t][:],
                        scalar1=EPS,
                        scalar2=rec[:],
                        op0=ALU.add,
                        op1=ALU.mult,
                    )
                ct = [
                    samp_pool.tile([128, L], bf16, tag="ct", bufs=4, name=f"ct_{s}_{i}")
                    for i in range(2)
                ]
                for jt in range(2):
                    for it in range(2):
                        tp = ps_xt.tile(
                            [128, 128], bf16, tag="xt", name=f"tp_{s}_{jt}_{it}"
                        )
                        nc.tensor.transpose(
                            tp[:], c_nat[it][:, 128 * jt : 128 * (jt + 1)], ident_bf[:]
                        )
                        nc.vector.tensor_copy(ct[jt][:, 128 * it : 128 * (it + 1)], tp[:])

                # ============ stream f: MLP + C ============================
                scores_nat = [
                    samp_pool.tile([128, L], f32, tag="snat", name=f"snat_{s}_{i}")
                    for i in range(2)
                ]
                C_t = [
                    samp_pool.tile([128, L], bf16, tag="C", name=f"C_{s}_{i}")
                    for i in range(2)
                ]

                def load_A(r, c):
                    # split in col-halves so downstream MLP groups can start
                    # as soon as the first 4 MiB lands
                    t = blk_pool.tile(
                        [128, 128, H], bf16, tag="blk", name=f"A_{s}_{r}_{c}"
                    )
                    for jh in range(4):
                        nc.gpsimd.dma_start(
                            out=t[:, 32 * jh : 32 * (jh + 1), :],
                            in_=f_in[
                                s,
                                128 * r : 128 * (r + 1),
                                128 * c + 32 * jh : 128 * c + 32 * (jh + 1),
                                :,
                            ],
                        )
                    return t

                def load_B(r, c):
                    # B[p, jj, h] = f[128c+jj, 128r+p, h] (strided direct
                    # load; split in jj-halves to stay under the 16384
                    # descriptor-per-DMA limit)
                    t = blk_pool.tile(
                        [128, 128, H], bf16, tag="blk", name=f"B_{s}_{r}_{c}"
                    )
                    for jh in range(4):
                        src = f_in[
                            s,
                            128 * c + 32 * jh : 128 * c + 32 * (jh + 1),
                            128 * r : 128 * (r + 1),
                            :,
                        ].rearrange("j p h -> p j h")
                        nc.gpsimd.dma_start(
                            out=t[:, 32 * jh : 32 * (jh + 1), :], in_=src
                        )
                    return t

                def shuffle_B_diag(src_tile, r):
                    # diagonal partner layout from the already-loaded A(r,r):
                    # per-row SBUF->SBUF DMAs on the otherwise-idle HWDGE,
                    # saving the 8 MiB HBM re-read of the block
                    t = blk_pool.tile(
                        [128, 128, H], bf16, tag="blk", name=f"B_{s}_{r}_{r}"
                    )
                    for jj in range(128):
                        nc.sync.dma_start(
                            out=t[:, jj, :], in_=src_tile[jj : jj + 1, :, :]
                        )
                    return t

                def mlp_block(blk, rt, cl, transposed_out=None):
                    # blk[:, jj, :] = 128 arcs; for A(r,c) these are
                    # (row 128r+p, col 128c+jj) -> scores_nat[rt] col chunks.
                    # For B(0,1) (= block10 data) they are (row 128+jj, col p)
                    # -> transposed staging, fixed up by one PE transpose.
                    s2_ps = None
                    for g in range(32):
                        xt_ps = ps_xt.tile(
                            [128, 512], bf16, tag="xt", name=f"xtps_{s}_{rt}_{cl}_{g}"
                        )
                        for t in range(4):
                            jj = 4 * g + t
                            nc.tensor.transpose(
                                xt_ps[:, 128 * t : 128 * (t + 1)],
                                blk[:, jj, :],
                                ident_bf[:],
                            )
                        xt_sb = xt_pool.tile(
                            [128, 512], bf16, tag="xt_sb", name=f"xtsb_{s}_{rt}_{cl}_{g}"
                        )
                        if g % 2 == 0:
                            nc.vector.tensor_copy(xt_sb[:], xt_ps[:])
                        else:
                            nc.scalar.copy(xt_sb[:], xt_ps[:])
                        hdn_ps = ps_mm.tile(
                            [128, 512], f32, tag="hdn", name=f"hdnps_{s}_{rt}_{cl}_{g}"
                        )
                        nc.tensor.matmul(hdn_ps[:], w1_bf[:], xt_sb[:])
                        hdn_sb = hdn_pool.tile(
                            [128, 512], bf16, tag="hdn_sb",
                            name=f"hdnsb_{s}_{rt}_{cl}_{g}",
                        )
                        nc.scalar.activation(hdn_sb[:], hdn_ps[:], AF.Tanh, bias=b1_sb[:])
                        if g % 8 == 0:
                            s2_ps = ps_s2.tile(
                                [128, 32], f32, tag="s2", name=f"s2ps_{s}_{rt}_{cl}_{g}"
                            )
                        for t in range(4):
                            jj = 4 * g + t
                            nc.tensor.matmul(
                                s2_ps[:, jj % 32 : jj % 32 + 1],
                                hdn_sb[:, 128 * t : 128 * (t + 1)],
                                w2_bf[:],
                            )
                        if g % 8 == 7:
                            q = g // 8
                            if transposed_out is None:
                                nc.vector.tensor_scalar_add(
                                    scores_nat[rt][
                                        :, 128 * cl + 32 * q : 128 * cl + 32 * (q + 1)
                                    ],
                                    s2_ps[:],
                                    b2_bcast[:],
                                )
                            else:
                                nc.vector.tensor_copy(
                                    transposed_out[:, 32 * q : 32 * (q + 1)], s2_ps[:]
                                )

                def c_compute(rt, col0, A, B):
                    for q in range(8):
                        sl = slice(16 * q, 16 * (q + 1))
                        prod = prod_pool.tile(
                            [128, 16, H], bf16, tag="prod",
                            name=f"prod_{s}_{rt}_{col0}_{q}",
                        )
                        nc.vector.tensor_mul(prod[:], A[:, sl, :], B[:, sl, :])
                        with nc.allow_low_precision(
                            "C in bf16 validated: wloss err ~1e-3"
                        ):
                            nc.vector.tensor_reduce(
                                C_t[rt][:, col0 + 16 * q : col0 + 16 * (q + 1)],
                                prod[:],
                                axis=AX.X,
                                op=ALU.add,
                            )

                # ---- (0,0) ----
                A00 = load_A(0, 0)
                B00 = load_B(0, 0)
                mlp_block(A00, 0, 0)
                c_compute(0, 0, A00, B00)
                # ---- (0,1) + (1,0) ----
                A01 = load_A(0, 1)
                B01 = load_B(0, 1)
                mlp_block(A01, 0, 1)
                sT10 = samp_pool.tile([128, 128], f32, tag="sT10", name=f"sT10_{s}")
                mlp_block(B01, 1, 0, transposed_out=sT10)
                tpS = ps_xt.tile([128, 128], f32, tag="xt", name=f"tpS_{s}")
                nc.tensor.transpose(tpS[:], sT10[:], ident_f32[:])
                nc.vector.tensor_scalar_add(scores_nat[1][:, 0:128], tpS[:], b2_bcast[:])
                c_compute(0, 128, A01, B01)
                # C(1,0) = C(0,1)^T
                tp_c = ps_xt.tile([128, 128], bf16, tag="xt", name=f"tpc_{s}")
                nc.tensor.transpose(tp_c[:], C_t[0][:, 128:256], ident_bf[:])
                nc.vector.tensor_copy(C_t[1][:, 0:128], tp_c[:])
                # ---- (1,1) ----
                A11 = load_A(1, 1)
                B11 = load_B(1, 1)
                mlp_block(A11, 1, 1)
                c_compute(1, 128, A11, B11)

                # ============ scores output ================================
                for it in range(2):
                    nc.sync.dma_start(
                        out=scores_out[s, 128 * it : 128 * (it + 1), :],
                        in_=scores_nat[it][:],
                    )

                # ============ softmax (natural) + transpose to Rt ==========
                R_nat = [
                    samp_pool.tile([128, L], bf16, tag="Rnat", name=f"Rnat_{s}_{i}")
                    for i in range(2)
                ]
                for it in range(2):
                    mxr = small_pool.tile([128, 1], f32, tag="mxr", name=f"mxr_{s}_{it}")
                    nc.vector.tensor_reduce(
                        mxr[:], scores_nat[it][:], axis=AX.X, op=ALU.max
                    )
                    nmxr = small_pool.tile([128, 1], f32, tag="nmxr", name=f"nmxr_{s}_{it}")
                    nc.vector.tensor_scalar_mul(nmxr[:], mxr[:], -1.0)
                    e_nat = small_pool.tile([128, L], f32, tag="enat", name=f"enat_{s}_{it}")
                    zs = small_pool.tile([128, 1], f32, tag="zs", name=f"zs_{s}_{it}")
                    nc.scalar.activation(
                        e_nat[:], scores_nat[it][:], AF.Exp, bias=nmxr[:], accum_out=zs[:]
                    )
                    zrec = small_pool.tile([128, 1], f32, tag="zrec", name=f"zrec_{s}_{it}")
                    nc.vector.reciprocal(zrec[:], zs[:])
                    nc.vector.tensor_scalar_mul(R_nat[it][:], e_nat[:], zrec[:])
                Rt = [
                    samp_pool.tile([128, L], bf16, tag="Rt", bufs=4, name=f"Rt_{s}_{i}")
                    for i in range(2)
                ]
                for jt in range(2):
                    for it in range(2):
                        tpr = ps_xt.tile(
                            [128, 128], bf16, tag="xt", name=f"tpr_{s}_{jt}_{it}"
                        )
                        nc.tensor.transpose(
                            tpr[:], R_nat[it][:, 128 * jt : 128 * (jt + 1)], ident_bf[:]
                        )
                        nc.vector.tensor_copy(Rt[jt][:, 128 * it : 128 * (it + 1)], tpr[:])

                # ============ K, KM ========================================
                mx = small_pool.tile([128, 1], f32, tag="mx", name=f"mx_{s}")
                mx2 = small_pool.tile([128, 1], f32, tag="mx2", name=f"mx2_{s}")
                nc.vector.tensor_reduce(
                    mx[:], C_t[0][:], axis=AX.X, op=ALU.max, apply_absolute_value=True
                )
                nc.vector.tensor_reduce(
                    mx2[:], C_t[1][:], axis=AX.X, op=ALU.max, apply_absolute_value=True
                )
                mxc = small_pool.tile([128, 1], f32, tag="mxc", name=f"mxc_{s}")
                nc.vector.tensor_max(mxc[:], mx[:], mx2[:])
                mxt = ps_xt.tile([1, 128], f32, tag="xt", name=f"mxt_{s}")
                nc.tensor.transpose(mxt[:], mxc[:], ident_f32[:])
                mxs = small_pool.tile([1, 1], f32, tag="mxs", name=f"mxs_{s}")
                nc.vector.tensor_reduce(mxs[:], mxt[:], axis=AX.X, op=ALU.max)
                mxe = small_pool.tile([1, 1], f32, tag="mxe", name=f"mxe_{s}")
                nc.vector.tensor_scalar_add(mxe[:], mxs[:], EPS)
                inv = small_pool.tile([1, 1], f32, tag="inv", name=f"inv_{s}")
                nc.vector.reciprocal(inv[:], mxe[:])
                sk = small_pool.tile([1, 1], f32, tag="sk", name=f"sk_{s}")
                nc.scalar.mul(sk[:], inv[:], -LAM)
                inv_b = small_pool.tile([128, 1], f32, tag="inv_b", name=f"invb_{s}")
                inv_ps = ps_xt.tile([128, 1], f32, tag="xt", name=f"invps_{s}")
                nc.tensor.matmul(inv_ps[:], ones_row_f32[:], inv[:])
                nc.vector.tensor_copy(inv_b[:], inv_ps[:])
                sk_b = small_pool.tile([128, 1], f32, tag="sk_b", name=f"skb_{s}")
                sk_ps = ps_xt.tile([128, 1], f32, tag="xt", name=f"skps2_{s}")
                nc.tensor.matmul(sk_ps[:], ones_row_f32[:], sk[:])
                nc.vector.tensor_copy(sk_b[:], sk_ps[:])

                K_t = [
                    samp_pool.tile([128, L], bf16, tag="K", bufs=4, name=f"K_{s}_{i}")
                    for i in range(2)
                ]
                KM_t = [
                    samp_pool.tile([128, L], bf16, tag="KM", bufs=4, name=f"KM_{s}_{i}")
                    for i in range(2)
                ]
                for t in range(2):
                    nc.scalar.activation(K_t[t][:], C_t[t][:], AF.Exp, scale=sk_b[:])
                    nc.vector.scalar_tensor_tensor(
                        out=KM_t[t][:],
                        in0=K_t[t][:],
                        scalar=inv_b[:],
                        in1=C_t[t][:],
                        op0=ALU.mult,
                        op1=ALU.mult,
                    )

                # ============ Sinkhorn state (loop runs after both samples) =
                Ut = [
                    samp_pool.tile([128, L], bf16, tag="Ut", bufs=4, name=f"Ut_{s}_{i}")
                    for i in range(2)
                ]
                for t in range(2):
                    nc.vector.memset(Ut[t][:], 1.0 / L)
                Bt = [
                    samp_pool.tile([128, L], bf16, tag="Bt", bufs=4, name=f"Bt_{s}_{i}")
                    for i in range(2)
                ]
                sink_state.append(dict(ct=ct, Rt=Rt, K_t=K_t, KM_t=KM_t, Ut=Ut, Bt=Bt))

            # ============ Sinkhorn: both samples interleaved ================
            def half_step(s, dst_name, numer_name, it_tag):
                st = sink_state[s]
                dst, numer = st[dst_name], st[numer_name]
                rhs_tiles = st["Ut"] if dst_name == "Bt" else st["Bt"]
                lhs_tiles = st["K_t"]
                for hf in range(2):
                    ps = ps_mm.tile(
                        [128, L], f32, tag="hdn", name=f"skps_{s}_{it_tag}_{hf}"
                    )
                    nc.tensor.matmul(
                        ps[:], eps_row_bf[:], ones_row_bf[:], start=True, stop=False
                    )
                    nc.tensor.matmul(
                        ps[:],
                        lhs_tiles[0][:, 128 * hf : 128 * (hf + 1)],
                        rhs_tiles[0][:],
                        start=False,
                        stop=False,
                    )
                    nc.tensor.matmul(
                        ps[:],
                        lhs_tiles[1][:, 128 * hf : 128 * (hf + 1)],
                        rhs_tiles[1][:],
                        start=False,
                        stop=True,
                    )
                    rcp = small_pool.tile(
                        [128, L], f32, tag="rcp", name=f"rcp_{s}_{it_tag}_{hf}", bufs=4
                    )
                    nc.vector.reciprocal_approx_fast(rcp[:], ps[:])
                    nc.vector.tensor_mul(dst[hf][:], numer[hf][:], rcp[:])

            for it_ in range(N_ITERS):
                for s in range(S_PER_CORE):
                    half_step(s, "Bt", "ct", 2 * it_)
                for s in range(S_PER_CORE):
                    half_step(s, "Ut", "Rt", 2 * it_ + 1)

            for s in range(S_PER_CORE):
                half_step(s, "Bt", "ct", 99)  # Bt = Vt
            for s in range(S_PER_CORE):
                st = sink_state[s]
                KM_t, Bt, Ut = st["KM_t"], st["Bt"], st["Ut"]
                for hf in range(2):
                    ps = ps_mm.tile([128, L], f32, tag="hdn", name=f"pt_{s}_{hf}")
                    nc.tensor.matmul(
                        ps[:], KM_t[0][:, 128 * hf : 128 * (hf + 1)], Bt[0][:],
                        start=True, stop=False,
                    )
                    nc.tensor.matmul(
                        ps[:], KM_t[1][:, 128 * hf : 128 * (hf + 1)], Bt[1][:],
                        start=False, stop=True,
                    )
                    junk = small_pool.tile([128, L], bf16, tag="junk", name=f"junk_{s}_{hf}")
                    nc.vector.scalar_tensor_tensor(
                        out=junk[:],
                        in0=Ut[hf][:],
                        scalar=1.0,
                        in1=ps[:],
                        op0=ALU.mult,
                        op1=ALU.mult,
                        accum_out=lossc[:, 2 * s + hf : 2 * s + hf + 1],
                    )

            # ---- total wloss ----
            wl_ps = ps_s2.tile([1, 2 * S_PER_CORE], f32, tag="s2", name="wl_ps")
            nc.tensor.matmul(wl_ps[:], ones_f32[:], lossc[:])
            wl_sb = const_pool.tile([1, 1], f32)
            nc.vector.tensor_reduce(wl_sb[:], wl_ps[:], axis=AX.X, op=ALU.add)
            wl_out_sb = const_pool.tile([1, 1], f32)
            nc.vector.tensor_copy(wl_out_sb[:], wl_sb[:])
            nc.sync.dma_start(out=wloss_out[:, :], in_=wl_out_sb[:])

    nc.finalize()
    return nc


_NC_CACHE = None


def _get_nc():
    global _NC_CACHE
    if _NC_CACHE is None:
        _NC_CACHE = build_nc()
    return _NC_CACHE


LAST_EXEC_NS = None


def kernel(f, head, W1, b1, W2, b2):
    f = np.ascontiguousarray(f, dtype=np.float32)
    head = np.ascontiguousarray(head, dtype=np.float32)
    W1 = np.ascontiguousarray(W1, dtype=np.float32)
    b1 = np.ascontiguousarray(b1, dtype=np.float32)
    W2 = np.ascontiguousarray(W2, dtype=np.float32)
    b2 = np.ascontiguousarray(b2, dtype=np.float32)

    from concourse.bass_utils import run_bass_kernel_spmd

    nc = _get_nc()
    in_maps = []
    for cid in range(N_CORES):
        s0 = cid * S_PER_CORE
        in_maps.append(
            {
                "f": f[s0 : s0 + S_PER_CORE],
                "head": head[s0 : s0 + S_PER_CORE],
                "W1": W1,
                "b1": b1,
                "W2": W2,
                "b2": b2,
            }
        )
    trace = bool(int(os.environ.get("ARC_KERNEL_TRACE", "0")))
    res = run_bass_kernel_spmd(nc, in_maps, list(range(N_CORES)), trace=trace)
    if trace:
        global LAST_EXEC_NS
        LAST_EXEC_NS = res.exec_time_ns
    scores = np.concatenate(
        [np.asarray(r["scores"]).reshape(S_PER_CORE, L, L) for r in res.results],
        axis=0,
    )
    wloss = np.float32(
        sum(float(np.asarray(r["wloss"]).reshape(-1)[0]) for r in res.results)
    )
    return scores, np.asarray(wloss, dtype=np.float32)


# revision 33
# speedup vs baseline: 1.8554x; 1.0538x over previous
"""Trainium2 Bass kernel for nn_ArcPredictorWloss.

Reference computation (per sample s of n=16, l=256, h=hid=128):
  scores = tanh(f.reshape(l*l, h) @ W1 + b1) @ W2 + b2          # (l, l)
  C[i,j] = sum_h f[i,j,h] * f[j,i,h]   (symmetric)
  Cn = C / (max|C| + eps)
  r = softmax(scores + eps, axis=-1);  c = (head+eps)/sum(head+eps, -1)
  per-row Sinkhorn (20 iters) with K = exp(-20*Cn), KM = K*Cn
  wloss = sum over rows/samples of u . (KM v)

Sharding: data-parallel over n across 8 cores (2 samples per core).
All heavy compute in bf16 (validated: scores err ~3.5e-3 rel-to-max,
wloss err ~1.2e-3 worst-case vs fp64), fp32 PSUM accumulation.

Layout strategy per sample (l=256 -> 2x2 grid of 128x128 blocks):
  - A(r,c)[p,jj,h] = f[128r+p, 128c+jj, h]   (natural, row-partition)
  - B(r,c)[p,jj,h] = f[128c+jj, 128r+p, h]   (partner, row-partition)
    Both loaded straight from HBM by casting gpsimd DMAs (fp32->bf16).
    Loaded: A00,B00,A01,B01,A11,B11 - block10's bytes arrive as B01, so
    off-diagonal data is read once; diagonal blocks are read twice.
  - C(r,c) = sum_h A(r,c)*B(r,c): multiply on GPSIMD (otherwise idle),
    reduce on DVE.  C(1,0) = C(0,1)^T via PE transpose.
  - MLP: PE-transpose of [128 rows, h] col-tiles -> Xt, stage-1 vs W1,
    tanh on ACT (PSUM->SBUF), stage-2 as [arcs,1] columns (lhsT=hdn
    tile, rhs=W2) accumulated into [128,32] PSUM chunks -> scores in
    natural layout (block10 via B01 lands transposed; one extra PE
    transpose fixes it up).
  - softmax over free dim; Sinkhorn state transposed [bin, row]; K/KM
    symmetric so the loop is matmul + approx-reciprocal + multiply.
    The +EPS inside the loop is a rank-1 (eps-row x ones-row)
    PSUM-accumulated matmul.  Both samples' Sinkhorn chains are emitted
    interleaved so the two dependency chains overlap on the engines.
"""

import os
import sys
import numpy as np

sys.path.insert(0, "/opt/trn_rl_repo")

LAM = 20.0
N_ITERS = 20
EPS = 1e-8

N, L, H = 16, 256, 128
N_CORES = 8
S_PER_CORE = N // N_CORES  # 2


def build_nc():
    import concourse.bass as bass
    import concourse.mybir as mybir
    from concourse import bacc, masks
    from concourse.tile import TileContext

    f32 = mybir.dt.float32
    bf16 = mybir.dt.bfloat16
    AF = mybir.ActivationFunctionType
    ALU = mybir.AluOpType
    AX = mybir.AxisListType

    nc = bacc.Bacc("TRN2", target_bir_lowering=False)

    f_in = nc.declare_dram_parameter("f", [S_PER_CORE, L, L, H], f32, isOutput=False)
    head_in = nc.declare_dram_parameter("head", [S_PER_CORE, L, L], f32, isOutput=False)
    w1_in = nc.declare_dram_parameter("W1", [H, H], f32, isOutput=False)
    b1_in = nc.declare_dram_parameter("b1", [H], f32, isOutput=False)
    w2_in = nc.declare_dram_parameter("W2", [H, 1], f32, isOutput=False)
    b2_in = nc.declare_dram_parameter("b2", [1], f32, isOutput=False)
    scores_out = nc.declare_dram_parameter(
        "scores", [S_PER_CORE, L, L], f32, isOutput=True
    )
    wloss_out = nc.declare_dram_parameter("wloss", [1, 1], f32, isOutput=True)

    with TileContext(nc) as tc:
        from contextlib import ExitStack

        ctx = ExitStack()
        with ctx:
            const_pool = ctx.enter_context(tc.tile_pool(name="const", bufs=1))
            blk_pool = ctx.enter_context(tc.tile_pool(name="blk", bufs=4))
            xt_pool = ctx.enter_context(tc.tile_pool(name="xt", bufs=5))
            hdn_pool = ctx.enter_context(tc.tile_pool(name="hdn", bufs=5))
            prod_pool = ctx.enter_context(tc.tile_pool(name="prod", bufs=3))
            small_pool = ctx.enter_context(tc.tile_pool(name="small", bufs=2))
            samp_pool = ctx.enter_context(tc.tile_pool(name="samp", bufs=2))
            ps_xt = ctx.enter_context(tc.tile_pool(name="ps_xt", bufs=3, space="PSUM"))
            ps_mm = ctx.enter_context(tc.tile_pool(name="ps_mm", bufs=3, space="PSUM"))
            ps_s2 = ctx.enter_context(tc.tile_pool(name="ps_s2", bufs=2, space="PSUM"))

            # ---- constants / weights ----
            ident_bf = const_pool.tile([128, 128], bf16)
            masks.make_identity(nc, ident_bf[:])
            ident_f32 = const_pool.tile([128, 128], f32)
            masks.make_identity(nc, ident_f32[:])
            ones_f32 = const_pool.tile([128, 1], f32)
            nc.vector.memset(ones_f32[:], 1.0)
            ones_row_f32 = const_pool.tile([1, 128], f32)
            nc.vector.memset(ones_row_f32[:], 1.0)
            eps_row_bf = const_pool.tile([1, 128], bf16)
            nc.vector.memset(eps_row_bf[:], EPS)
            ones_row_bf = const_pool.tile([1, L], bf16)
            nc.vector.memset(ones_row_bf[:], 1.0)
            eps_col = const_pool.tile([128, 1], f32)
            nc.vector.memset(eps_col[:], EPS)

            w1_f32 = const_pool.tile([H, H], f32)
            nc.sync.dma_start(out=w1_f32[:], in_=w1_in[:, :])
            w1_bf = const_pool.tile([H, H], bf16)
            nc.vector.tensor_copy(w1_bf[:], w1_f32[:])

            b1_sb = const_pool.tile([H, 1], f32)
            nc.sync.dma_start(
                out=b1_sb[:], in_=b1_in[:].rearrange("(h one) -> h one", one=1)
            )

            w2_f32 = const_pool.tile([H, 1], f32)
            nc.sync.dma_start(out=w2_f32[:], in_=w2_in[:, :])
            w2_bf = const_pool.tile([H, 1], bf16)
            nc.vector.tensor_copy(w2_bf[:], w2_f32[:])

            b2_sb = const_pool.tile([1, 1], f32)
            nc.sync.dma_start(
                out=b2_sb[:], in_=b2_in[:].rearrange("(o one) -> o one", one=1)
            )
            b2_bcast = const_pool.tile([128, 1], f32)
            nc.sync.dma_start(
                out=b2_bcast[:],
                in_=b2_in[:]
                .rearrange("(o one) -> o one", one=1)
                .to_broadcast((128, 1)),
            )

            # per-(sample,half) loss partials
            lossc = const_pool.tile([128, 2 * S_PER_CORE], f32)
            sink_state = []

            for s in range(S_PER_CORE):
                # ============ head -> ct (transposed target hist) ==========
                head_nat = [
                    samp_pool.tile([128, L], f32, tag="head", name=f"head_{s}_{i}")
                    for i in range(2)
                ]
                c_nat = [
                    samp_pool.tile([128, L], bf16, tag="cnat", name=f"cnat_{s}_{i}")
                    for i in range(2)
                ]
                for t in range(2):
                    nc.sync.dma_start(
                        out=head_nat[t][:], in_=head_in[s, 128 * t : 128 * (t + 1), :]
                    )
                    rs = small_pool.tile([128, 1], f32, tag="rs", name=f"rs_{s}_{t}")
                    nc.vector.tensor_reduce(rs[:], head_nat[t][:], axis=AX.X, op=ALU.add)
                    rs_eps = small_pool.tile(
                        [128, 1], f32, tag="rs_eps", name=f"rse_{s}_{t}"
                    )
                    nc.vector.tensor_scalar_add(rs_eps[:], rs[:], float(L) * EPS)
                    rec = small_pool.tile([128, 1], f32, tag="rec", name=f"rec_{s}_{t}")
                    nc.vector.reciprocal(rec[:], rs_eps[:])
                    # c = (head + eps) * (1/sum) in one DVE pass
                    nc.vector.tensor_scalar(
                        out=c_nat[t][:],
                        in0=head_nat[t][:],
                        scalar1=EPS,
                        scalar2=rec[:],
                        op0=ALU.add,
                        op1=ALU.mult,
                    )
                ct = [
                    samp_pool.tile([128, L], bf16, tag="ct", bufs=4, name=f"ct_{s}_{i}")
                    for i in range(2)
                ]
                for jt in range(2):
                    for it in range(2):
                        tp = ps_xt.tile(
                            [128, 128], bf16, tag="xt", name=f"tp_{s}_{jt}_{it}"
                        )
                        nc.tensor.transpose(
                            tp[:], c_nat[it][:, 128 * jt : 128 * (jt + 1)], ident_bf[:]
                        )
                        nc.vector.tensor_copy(ct[jt][:, 128 * it : 128 * (it + 1)], tp[:])

                # ============ stream f: MLP + C ============================
                scores_nat = [
                    samp_pool.tile([128, L], f32, tag="snat", name=f"snat_{s}_{i}")
                    for i in range(2)
                ]
                C_t = [
                    samp_pool.tile([128, L], bf16, tag="C", name=f"C_{s}_{i}")
                    for i in range(2)
                ]

                def load_A(r, c):
                    # split in col-halves so downstream MLP groups can start
                    # as soon as the first 4 MiB lands
                    t = blk_pool.tile(
                        [128, 128, H], bf16, tag="blk", name=f"A_{s}_{r}_{c}"
                    )
                    for jh in range(4):
                        nc.gpsimd.dma_start(
                            out=t[:, 32 * jh : 32 * (jh + 1), :],
                            in_=f_in[
                                s,
                                128 * r : 128 * (r + 1),
                                128 * c + 32 * jh : 128 * c + 32 * (jh + 1),
                                :,
                            ],
                        )
                    return t

                def load_B(r, c):
                    # B[p, jj, h] = f[128c+jj, 128r+p, h] (strided direct
                    # load; split in jj-halves to stay under the 16384
                    # descriptor-per-DMA limit)
                    t = blk_pool.tile(
                        [128, 128, H], bf16, tag="blk", name=f"B_{s}_{r}_{c}"
                    )
                    for jh in range(4):
                        src = f_in[
                            s,
                            128 * c + 32 * jh : 128 * c + 32 * (jh + 1),
                            128 * r : 128 * (r + 1),
                            :,
                        ].rearrange("j p h -> p j h")
                        nc.gpsimd.dma_start(
                            out=t[:, 32 * jh : 32 * (jh + 1), :], in_=src
                        )
                    return t

                def shuffle_B_diag(src_tile, r):
                    # diagonal partner layout from the already-loaded A(r,r):
                    # per-row SBUF->SBUF DMAs on the otherwise-idle HWDGE,
                    # saving the 8 MiB HBM re-read of the block
                    t = blk_pool.tile(
                        [128, 128, H], bf16, tag="blk", name=f"B_{s}_{r}_{r}"
                    )
                    for jj in range(128):
                        nc.sync.dma_start(
                            out=t[:, jj, :], in_=src_tile[jj : jj + 1, :, :]
                        )
                    return t

                def mlp_block(blk, rt, cl, transposed_out=None):
                    # blk[:, jj, :] = 128 arcs; for A(r,c) these are
                    # (row 128r+p, col 128c+jj) -> scores_nat[rt] col chunks.
                    # For B(0,1) (= block10 data) they are (row 128+jj, col p)
                    # -> transposed staging, fixed up by one PE transpose.
                    s2_ps = None
                    for g in range(32):
                        xt_ps = ps_xt.tile(
                            [128, 512], bf16, tag="xt", name=f"xtps_{s}_{rt}_{cl}_{g}"
                        )
                        for t in range(4):
                            jj = 4 * g + t
                            nc.tensor.transpose(
                                xt_ps[:, 128 * t : 128 * (t + 1)],
                                blk[:, jj, :],
                                ident_bf[:],
                            )
                        xt_sb = xt_pool.tile(
                            [128, 512], bf16, tag="xt_sb", name=f"xtsb_{s}_{rt}_{cl}_{g}"
                        )
                        if g % 2 == 0:
                            nc.vector.tensor_copy(xt_sb[:], xt_ps[:])
                        else:
                            nc.scalar.copy(xt_sb[:], xt_ps[:])
                        hdn_ps = ps_mm.tile(
                            [128, 512], f32, tag="hdn", name=f"hdnps_{s}_{rt}_{cl}_{g}"
                        )
                        nc.tensor.matmul(hdn_ps[:], w1_bf[:], xt_sb[:])
                        hdn_sb = hdn_pool.tile(
                            [128, 512], bf16, tag="hdn_sb",
                            name=f"hdnsb_{s}_{rt}_{cl}_{g}",
                        )
                        nc.scalar.activation(hdn_sb[:], hdn_ps[:], AF.Tanh, bias=b1_sb[:])
                        if g % 8 == 0:
                            s2_ps = ps_s2.tile(
                                [128, 32], f32, tag="s2", name=f"s2ps_{s}_{rt}_{cl}_{g}"
                            )
                        for t in range(4):
                            jj = 4 * g + t
                            nc.tensor.matmul(
                                s2_ps[:, jj % 32 : jj % 32 + 1],
                                hdn_sb[:, 128 * t : 128 * (t + 1)],
                                w2_bf[:],
                            )
                        if g % 8 == 7:
                            q = g // 8
                            if transposed_out is None:
                                nc.vector.tensor_scalar_add(
                                    scores_nat[rt][
                                        :, 128 * cl + 32 * q : 128 * cl + 32 * (q + 1)
                                    ],
                                    s2_ps[:],
                                    b2_bcast[:],
                                )
                            else:
                                nc.vector.tensor_copy(
                                    transposed_out[:, 32 * q : 32 * (q + 1)], s2_ps[:]
                                )

                def c_compute(rt, col0, A, B):
                    for q in range(8):
                        sl = slice(16 * q, 16 * (q + 1))
                        prod = prod_pool.tile(
                            [128, 16, H], bf16, tag="prod",
                            name=f"prod_{s}_{rt}_{col0}_{q}",
                        )
                        nc.vector.tensor_mul(prod[:], A[:, sl, :], B[:, sl, :])
                        with nc.allow_low_precision(
                            "C in bf16 validated: wloss err ~1e-3"
                        ):
                            nc.vector.tensor_reduce(
                                C_t[rt][:, col0 + 16 * q : col0 + 16 * (q + 1)],
                                prod[:],
                                axis=AX.X,
                                op=ALU.add,
                            )

                # ---- (0,0) ----
                A00 = load_A(0, 0)
                B00 = load_B(0, 0)
                mlp_block(A00, 0, 0)
                c_compute(0, 0, A00, B00)
                # ---- (0,1) + (1,0) ----
                A01 = load_A(0, 1)
                B01 = load_B(0, 1)
                mlp_block(A01, 0, 1)
                sT10 = samp_pool.tile([128, 128], f32, tag="sT10", name=f"sT10_{s}")
                mlp_block(B01, 1, 0, transposed_out=sT10)
                tpS = ps_xt.tile([128, 128], f32, tag="xt", name=f"tpS_{s}")
                nc.tensor.transpose(tpS[:], sT10[:], ident_f32[:])
                nc.vector.tensor_scalar_add(scores_nat[1][:, 0:128], tpS[:], b2_bcast[:])
                c_compute(0, 128, A01, B01)
                # C(1,0) = C(0,1)^T
                tp_c = ps_xt.tile([128, 128], bf16, tag="xt", name=f"tpc_{s}")
                nc.tensor.transpose(tp_c[:], C_t[0][:, 128:256], ident_bf[:])
                nc.vector.tensor_copy(C_t[1][:, 0:128], tp_c[:])
                # ---- (1,1) ----
                A11 = load_A(1, 1)
                B11 = load_B(1, 1)
                mlp_block(A11, 1, 1)
                c_compute(1, 128, A11, B11)

                # ============ scores output ================================
                for it in range(2):
                    nc.sync.dma_start(
                        out=scores_out[s, 128 * it : 128 * (it + 1), :],
                        in_=scores_nat[it][:],
                    )

                # ============ softmax (natural) + transpose to Rt ==========
                R_nat = [
                    samp_pool.tile([128, L], bf16, tag="Rnat", name=f"Rnat_{s}_{i}")
                    for i in range(2)
                ]
                for it in range(2):
                    mxr = small_pool.tile([128, 1], f32, tag="mxr", name=f"mxr_{s}_{it}")
                    nc.vector.tensor_reduce(
                        mxr[:], scores_nat[it][:], axis=AX.X, op=ALU.max
                    )
                    nmxr = small_pool.tile([128, 1], f32, tag="nmxr", name=f"nmxr_{s}_{it}")
                    nc.vector.tensor_scalar_mul(nmxr[:], mxr[:], -1.0)
                    e_nat = small_pool.tile([128, L], f32, tag="enat", name=f"enat_{s}_{it}")
                    zs = small_pool.tile([128, 1], f32, tag="zs", name=f"zs_{s}_{it}")
                    nc.scalar.activation(
                        e_nat[:], scores_nat[it][:], AF.Exp, bias=nmxr[:], accum_out=zs[:]
                    )
                    zrec = small_pool.tile([128, 1], f32, tag="zrec", name=f"zrec_{s}_{it}")
                    nc.vector.reciprocal(zrec[:], zs[:])
                    nc.vector.tensor_scalar_mul(R_nat[it][:], e_nat[:], zrec[:])
                Rt = [
                    samp_pool.tile([128, L], bf16, tag="Rt", bufs=4, name=f"Rt_{s}_{i}")
                    for i in range(2)
                ]
                for jt in range(2):
                    for it in range(2):
                        tpr = ps_xt.tile(
                            [128, 128], bf16, tag="xt", name=f"tpr_{s}_{jt}_{it}"
                        )
                        nc.tensor.transpose(
                            tpr[:], R_nat[it][:, 128 * jt : 128 * (jt + 1)], ident_bf[:]
                        )
                        nc.vector.tensor_copy(Rt[jt][:, 128 * it : 128 * (it + 1)], tpr[:])

                # ============ K, KM ========================================
                mx = small_pool.tile([128, 1], f32, tag="mx", name=f"mx_{s}")
                mx2 = small_pool.tile([128, 1], f32, tag="mx2", name=f"mx2_{s}")
                nc.vector.tensor_reduce(
                    mx[:], C_t[0][:], axis=AX.X, op=ALU.max, apply_absolute_value=True
                )
                nc.vector.tensor_reduce(
                    mx2[:], C_t[1][:], axis=AX.X, op=ALU.max, apply_absolute_value=True
                )
                mxc = small_pool.tile([128, 1], f32, tag="mxc", name=f"mxc_{s}")
                nc.vector.tensor_max(mxc[:], mx[:], mx2[:])
                mxt = ps_xt.tile([1, 128], f32, tag="xt", name=f"mxt_{s}")
                nc.tensor.transpose(mxt[:], mxc[:], ident_f32[:])
                mxs = small_pool.tile([1, 1], f32, tag="mxs", name=f"mxs_{s}")
                nc.vector.tensor_reduce(mxs[:], mxt[:], axis=AX.X, op=ALU.max)
                mxe = small_pool.tile([1, 1], f32, tag="mxe", name=f"mxe_{s}")
                nc.vector.tensor_scalar_add(mxe[:], mxs[:], EPS)
                inv = small_pool.tile([1, 1], f32, tag="inv", name=f"inv_{s}")
                nc.vector.reciprocal(inv[:], mxe[:])
                sk = small_pool.tile([1, 1], f32, tag="sk", name=f"sk_{s}")
                nc.scalar.mul(sk[:], inv[:], -LAM)
                inv_b = small_pool.tile([128, 1], f32, tag="inv_b", name=f"invb_{s}")
                inv_ps = ps_xt.tile([128, 1], f32, tag="xt", name=f"invps_{s}")
                nc.tensor.matmul(inv_ps[:], ones_row_f32[:], inv[:])
                nc.vector.tensor_copy(inv_b[:], inv_ps[:])
                sk_b = small_pool.tile([128, 1], f32, tag="sk_b", name=f"skb_{s}")
                sk_ps = ps_xt.tile([128, 1], f32, tag="xt", name=f"skps2_{s}")
                nc.tensor.matmul(sk_ps[:], ones_row_f32[:], sk[:])
                nc.vector.tensor_copy(sk_b[:], sk_ps[:])

                K_t = [
                    samp_pool.tile([128, L], bf16, tag="K", bufs=4, name=f"K_{s}_{i}")
                    for i in range(2)
                ]
                KM_t = [
                    samp_pool.tile([128, L], bf16, tag="KM", bufs=4, name=f"KM_{s}_{i}")
                    for i in range(2)
                ]
                for t in range(2):
                    nc.scalar.activation(K_t[t][:], C_t[t][:], AF.Exp, scale=sk_b[:])
                    nc.vector.scalar_tensor_tensor(
                        out=KM_t[t][:],
                        in0=K_t[t][:],
                        scalar=inv_b[:],
                        in1=C_t[t][:],
                        op0=ALU.mult,
                        op1=ALU.mult,
                    )

                # ============ Sinkhorn state (loop runs after both samples) =
                Ut = [
                    samp_pool.tile([128, L], bf16, tag="Ut", bufs=4, name=f"Ut_{s}_{i}")
                    for i in range(2)
                ]
                for t in range(2):
                    nc.vector.memset(Ut[t][:], 1.0 / L)
                Bt = [
                    samp_pool.tile([128, L], bf16, tag="Bt", bufs=4, name=f"Bt_{s}_{i}")
                    for i in range(2)
                ]
                sink_state.append(dict(ct=ct, Rt=Rt, K_t=K_t, KM_t=KM_t, Ut=Ut, Bt=Bt))

            # ============ Sinkhorn: both samples interleaved ================
            def half_step(s, dst_name, numer_name, it_tag):
                st = sink_state[s]
                dst, numer = st[dst_name], st[numer_name]
                rhs_tiles = st["Ut"] if dst_name == "Bt" else st["Bt"]
                lhs_tiles = st["K_t"]
                for hf in range(2):
                    ps = ps_mm.tile(
                        [128, L], f32, tag="hdn", name=f"skps_{s}_{it_tag}_{hf}"
                    )
                    nc.tensor.matmul(
                        ps[:],
                        lhs_tiles[0][:, 128 * hf : 128 * (hf + 1)],
                        rhs_tiles[0][:],
                        start=True,
                        stop=False,
                    )
                    nc.tensor.matmul(
                        ps[:],
                        lhs_tiles[1][:, 128 * hf : 128 * (hf + 1)],
                        rhs_tiles[1][:],
                        start=False,
                        stop=True,
                    )
                    aeps = small_pool.tile(
                        [128, L], f32, tag="aeps", name=f"aeps_{s}_{it_tag}_{hf}", bufs=4
                    )
                    nc.scalar.activation(aeps[:], ps[:], AF.Identity, bias=eps_col[:])
                    rcp = small_pool.tile(
                        [128, L], f32, tag="rcp", name=f"rcp_{s}_{it_tag}_{hf}", bufs=4
                    )
                    nc.vector.reciprocal_approx_fast(rcp[:], aeps[:])
                    nc.vector.tensor_mul(dst[hf][:], numer[hf][:], rcp[:])

            for it_ in range(N_ITERS):
                for s in range(S_PER_CORE):
                    half_step(s, "Bt", "ct", 2 * it_)
                for s in range(S_PER_CORE):
                    half_step(s, "Ut", "Rt", 2 * it_ + 1)

            for s in range(S_PER_CORE):
                half_step(s, "Bt", "ct", 99)  # Bt = Vt
            for s in range(S_PER_CORE):
                st = sink_state[s]
                KM_t, Bt, Ut = st["KM_t"], st["Bt"], st["Ut"]
                for hf in range(2):
                    ps = ps_mm.tile([128, L], f32, tag="hdn", name=f"pt_{s}_{hf}")
                    nc.tensor.matmul(
                        ps[:], KM_t[0][:, 128 * hf : 128 * (hf + 1)], Bt[0][:],
                        start=True, stop=False,
                    )
                    nc.tensor.matmul(
                        ps[:], KM_t[1][:, 128 * hf : 128 * (hf + 1)], Bt[1][:],
                        start=False, stop=True,
                    )
                    junk = small_pool.tile([128, L], bf16, tag="junk", name=f"junk_{s}_{hf}")
                    nc.vector.scalar_tensor_tensor(
                        out=junk[:],
                        in0=Ut[hf][:],
                        scalar=1.0,
                        in1=ps[:],
                        op0=ALU.mult,
                        op1=ALU.mult,
                        accum_out=lossc[:, 2 * s + hf : 2 * s + hf + 1],
                    )

            # ---- total wloss ----
            wl_ps = ps_s2.tile([1, 2 * S_PER_CORE], f32, tag="s2", name="wl_ps")
            nc.tensor.matmul(wl_ps[:], ones_f32[:], lossc[:])
            wl_sb = const_pool.tile([1, 1], f32)
            nc.vector.tensor_reduce(wl_sb[:], wl_ps[:], axis=AX.X, op=ALU.add)
            wl_out_sb = const_pool.tile([1, 1], f32)
            nc.vector.tensor_copy(wl_out_sb[:], wl_sb[:])
            nc.sync.dma_start(out=wloss_out[:, :], in_=wl_out_sb[:])

    nc.finalize()
    return nc


_NC_CACHE = None


def _get_nc():
    global _NC_CACHE
    if _NC_CACHE is None:
        _NC_CACHE = build_nc()
    return _NC_CACHE


LAST_EXEC_NS = None


def kernel(f, head, W1, b1, W2, b2):
    f = np.ascontiguousarray(f, dtype=np.float32)
    head = np.ascontiguousarray(head, dtype=np.float32)
    W1 = np.ascontiguousarray(W1, dtype=np.float32)
    b1 = np.ascontiguousarray(b1, dtype=np.float32)
    W2 = np.ascontiguousarray(W2, dtype=np.float32)
    b2 = np.ascontiguousarray(b2, dtype=np.float32)

    from concourse.bass_utils import run_bass_kernel_spmd

    nc = _get_nc()
    in_maps = []
    for cid in range(N_CORES):
        s0 = cid * S_PER_CORE
        in_maps.append(
            {
                "f": f[s0 : s0 + S_PER_CORE],
                "head": head[s0 : s0 + S_PER_CORE],
                "W1": W1,
                "b1": b1,
                "W2": W2,
                "b2": b2,
            }
        )
    trace = bool(int(os.environ.get("ARC_KERNEL_TRACE", "0")))
    res = run_bass_kernel_spmd(nc, in_maps, list(range(N_CORES)), trace=trace)
    if trace:
        global LAST_EXEC_NS
        LAST_EXEC_NS = res.exec_time_ns
    scores = np.concatenate(
        [np.asarray(r["scores"]).reshape(S_PER_CORE, L, L) for r in res.results],
        axis=0,
    )
    wloss = np.float32(
        sum(float(np.asarray(r["wloss"]).reshape(-1)[0]) for r in res.results)
    )
    return scores, np.asarray(wloss, dtype=np.float32)
